# revision 26
# baseline (speedup 1.0000x reference)
"""XNOR-Net BasicBlock (BN-sign-conv x2 + residual, training-mode BN) on 8 TRN2 cores.

Strategy (data-parallel on batch, 4 images/core):
  phase0: x streamed fp16 (gpsimd cast DMA, kept for the phase-3 residual);
          per-channel sum via DVE ts-accum, sumsq via ACT Square / DVE
          tensor_tensor_reduce -> AllGather (BN1 stats)
  conv1 : x re-streamed f32 (overlaps AR1) for exact s1 = sign(x - t1) fp8;
          3x3 conv as 9 DoubleRow fp8 matmuls per 8-row band into 4-band
          PSUM tiles; epilogue per 4/3-band chunk: DVE copy psum->u (fp16),
          in-place DVE stt u = max(y, a*y) with accum -> sum(u);
          sum(u^2): cc0 via ACT Square+accum, cc1 via GpSimd square +
          DVE ts-accum  -> AllGather (BN2 stats)
  conv2 : s2 = sign(u1 - theta2), same -> AllGather (BN3 stats)
  phase3: out = prelu(K*u2 + D + x, a3) in fp16 (host casts to f32)
"""

import sys

sys.path.insert(0, "/opt/trn_rl_repo")

import contextlib

import numpy as np

import concourse.bacc as bacc
import concourse.mybir as mybir
import concourse.tile as tile
from concourse.bass_utils import run_bass_kernel_spmd

F32 = mybir.dt.float32
F16 = mybir.dt.float16
BF16 = mybir.dt.bfloat16
F8 = mybir.dt.float8e4
F8NP = mybir.dt.np(F8)

AF = mybir.ActivationFunctionType
OP = mybir.AluOpType
DR = mybir.MatmulPerfMode.DoubleRow

NCORES = 8
B, C, H, W = 32, 256, 56, 56
BL = B // NCORES          # images per core
HW = H * W                # 3136
HHW = HW // 2             # 1568 half plane
PW = W + 2                # 58 padded width
PLANE = PW * PW           # 3364 padded plane (58 rows x 58 cols)
PLANE_PAD = 3392          # plane stride, %16 == 0
BAND = 8                  # output rows per matmul
NBAND = H // BAND         # 7
NFREE = BAND * PW         # 464 psum free size per band
BANK = 512                # psum band stride (one 2KB bank)
NTOT = B * HW             # BN count (N*H*W over full batch)
EPS = 1e-5
OFFS = [(dh, dw) for dh in range(3) for dw in range(3)]
CHUNKS = ((0, 2), (2, 2), (4, 2), (6, 1))  # (first band, nbands) psum chunks

NTOT_ = float(NTOT)

# cvec column indices
CV_B1, CV_A1, CV_SF1N, CV_SF1SQN, CV_B2, CV_ISF1, CV_A2, \
    CV_SF2N, CV_SF2SQN, CV_G3SF2, CV_G3, CV_B3V, CV_A3, CV_EPS = range(14)
CV_NCOLS = 14

_CACHE = {}


def _build():
    nc = bacc.Bacc(num_devices=NCORES)
    x_d = nc.declare_dram_parameter("x", [BL, C, H, W], F32, isOutput=False)
    w1_d = nc.declare_dram_parameter("w1s", [128, 2, 18 * 128], F8, isOutput=False)
    w2_d = nc.declare_dram_parameter("w2s", [128, 2, 18 * 128], F8, isOutput=False)
    cv_d = nc.declare_dram_parameter("cvec", [128, 2, CV_NCOLS], F32, isOutput=False)
    out_d = nc.declare_dram_parameter("out", [BL, C, H, W], F16, isOutput=True)

    # DRAM-side views: channel c -> (g = c // 128, p = c % 128)
    def x_view(n):
        return x_d[n].rearrange("(g p) h w -> p g (h w)", p=128)

    def out_view(n):
        return out_d[n].rearrange("(g p) h w -> p g (h w)", p=128)

    with tile.TileContext(nc, num_cores=NCORES, pool_alloc_mode="queue") as tc:
        es_u1 = contextlib.ExitStack()
        es_u2 = contextlib.ExitStack()
        es_xs = contextlib.ExitStack()
        with tc.tile_pool(name="consts", bufs=1) as cpool, \
                tc.tile_pool(name="weights", bufs=1) as wpool, \
                tc.tile_pool(name="spool", bufs=1) as spool, \
                tc.tile_pool(name="sqscr", bufs=1) as sqpool, \
                tc.tile_pool(name="psum", bufs=1, space="PSUM") as psum_pool, \
                tc.tile_pool(name="dram", bufs=1, space="DRAM") as dram_pool, \
                es_u2:

            # ---- persistent small tiles ----
            cvec = cpool.tile([128, 2, CV_NCOLS], F32, tag="cvec")
            st1 = cpool.tile([128, 32], F32, tag="st1")
            st2 = cpool.tile([128, 112], F32, tag="st2")
            st3 = cpool.tile([128, 112], F32, tag="st3")
            g1 = cpool.tile([128, 4], F32, tag="g1")
            g2 = cpool.tile([128, 4], F32, tag="g2")
            g3t = cpool.tile([128, 4], F32, tag="g3t")
            negt1 = cpool.tile([128, 2], F32, tag="negt1")
            negth2 = cpool.tile([128, 2], F32, tag="negth2")
            kvec = cpool.tile([128, 2], F32, tag="kvec")
            dvec = cpool.tile([128, 2], F32, tag="dvec")
            tmp_a = cpool.tile([128, 2], F32, tag="tmp_a")
            tmp_b = cpool.tile([128, 2], F32, tag="tmp_b")
            tmp_c = cpool.tile([128, 2], F32, tag="tmp_c")

            w1t = wpool.tile([128, 2, 18 * 128], F8, tag="w1t")
            w2t = wpool.tile([128, 2, 18 * 128], F8, tag="w2t")

            s_tiles = [
                spool.tile([128, 2, PLANE_PAD], F8, tag="sa", name="sa"),
                spool.tile([128, 2, PLANE_PAD], F8, tag="sb", name="sb"),
            ]

            # shared square scratch (garbage out): sqs for ACT squares,
            # sqb (half-size) for GpSimd/DVE squares — separate tags so the
            # two engines' square chains don't serialize on WAW deps
            sqs = sqpool.tile([128, HW], F16, tag="sqs")
            sqb = sqpool.tile([128, HHW], BF16, tag="sqb")

            # persistent fp16 x (phase0 stats source + phase3 residual), u2
            x16_pool = es_u2.enter_context(tc.tile_pool(name="x16", bufs=BL))
            u2_pool = es_u2.enter_context(tc.tile_pool(name="u2", bufs=BL))
            u1_pool = es_u1.enter_context(tc.tile_pool(name="u1", bufs=BL))

            # =============== phase 0: fp16 x stream + stats ===============
            # st1 col j = (k*2 + g)*8 + n*2 + h  (k: 0=sum, 1=sumsq; h=half)
            # cast DMAs first on the gpsimd queue so their descriptor gen
            # isn't stuck behind the big s-tile memsets
            # stats-tile memsets first on the pool queue: the stats accums
            # wait on these (WAW), so they must precede casts and s-memsets
            nc.gpsimd.memset(st1[:], 0.0)
            nc.gpsimd.memset(st2[:], 0.0)
            nc.gpsimd.memset(st3[:], 0.0)
            x16s = []
            for n in range(BL):
                x16 = x16_pool.tile([128, 2, HW], F16, tag="x16", name=f"x16_{n}")
                x16s.append(x16)
                for g in range(2):
                    # cast f32 -> fp16 in flight (gpsimd DGE)
                    nc.gpsimd.dma_start(x16[:, g, :], x_view(n)[:, g, :])

            nc.sync.dma_start(cvec[:], cv_d[:])
            nc.sync.dma_start(w1t[:], w1_d[:])
            nc.sync.dma_start(w2t[:], w2_d[:])
            for s in s_tiles:
                nc.gpsimd.memset(s[:], 0.0)

            def s_plane(s, g):
                return s[:, g, 0:PLANE].rearrange("p (r w) -> p r w", w=PW)

            for n in range(BL):
                x16 = x16s[n]
                for g in range(2):
                    # sum(x) via in-place ts-accum (4x DVE mode)
                    nc.vector.tensor_scalar(
                        x16[:, g, :], x16[:, g, :], 1.0, 0.0,
                        op0=OP.mult, op1=OP.add,
                        accum_out=st1[:, (0 * 2 + g) * 8 + n * 2:(0 * 2 + g) * 8 + n * 2 + 1],
                    )
                    jq = (1 * 2 + g) * 8 + n * 2
                    if n < 3:
                        # ACT square, exact f32 accum
                        nc.scalar.activation(
                            sqs[:], x16[:, g, :], AF.Square, bias=0.0, scale=1.0,
                            accum_out=st1[:, jq:jq + 1],
                        )
                    else:
                        # last image: ACT halves keep the stats tail short
                        for h in range(2):
                            xh = x16[:, g, h * HHW:(h + 1) * HHW]
                            nc.scalar.activation(
                                sqs[:, 0:HHW], xh, AF.Square, bias=0.0,
                                scale=1.0, accum_out=st1[:, jq + h:jq + h + 1],
                            )

            # reduce st1 [128, (k g) 8] -> r1 [128, 4], AllGather -> g1
            r1 = cpool.tile([128, 4], F32, tag="r1")
            nc.vector.reduce_sum(
                r1[:].rearrange("p (a b) -> p a b", b=1),
                st1[:].rearrange("p (kg t) -> p kg t", t=2 * BL),
                axis=mybir.AxisListType.X,
            )
            ar1_i = dram_pool.tile([128, 4], F32, tag="ar1_i")
            ar1_o = dram_pool.tile([NCORES, 128, 4], F32, tag="ar1_o", addr_space="Shared")
            nc.sync.dma_start(ar1_i[:], r1[:])
            nc.gpsimd.collective_compute(
                "AllGather", OP.bypass, replica_groups=[list(range(NCORES))],
                ins=[ar1_i[:].opt()], outs=[ar1_o[:].opt()],
            )
            # gth read on the ACT hwdge queue so SP can run the conv1 x
            # re-stream during the collective
            gth1 = cpool.tile([128, 4, NCORES], F32, tag="gth1")
            nc.scalar.dma_start(gth1[:], ar1_o[:].rearrange("r p k -> p k r"))
            nc.vector.reduce_sum(
                g1[:].rearrange("p (a b) -> p a b", b=1), gth1[:],
                axis=mybir.AxisListType.X,
            )

            # conv1 f32 x re-stream (per half-image); transfers overlap AR1
            xs_pool = es_xs.enter_context(tc.tile_pool(name="xs", bufs=2))
            x1s = []
            for n in range(BL):
                halves = []
                for g in range(2):
                    xt = xs_pool.tile([128, HW], F32, tag="xt", name=f"x1_{n}_{g}")
                    nc.sync.dma_start(xt[:], x_view(n)[:, g, :])
                    halves.append(xt)
                x1s.append(halves)

            # ---- BN1 threshold: negt1 = B1*std1 - m1 ----
            g1v = g1[:].rearrange("p (k g) -> p k g", k=2)
            nc.vector.tensor_scalar_mul(tmp_a[:], g1v[:, 0], 1.0 / NTOT_)     # m1
            nc.vector.tensor_scalar_mul(tmp_b[:], g1v[:, 1], 1.0 / NTOT_)     # E[x^2]
            nc.vector.scalar_tensor_tensor(                                   # -m^2
                tmp_c[:], tmp_a[:], -1.0, tmp_a[:], op0=OP.mult, op1=OP.mult,
            )
            nc.vector.tensor_add(tmp_c[:], tmp_c[:], tmp_b[:])                # v1
            nc.scalar.activation(tmp_b[:], tmp_c[:], AF.Sqrt, bias=cvec[:, 0, CV_EPS:CV_EPS + 1], scale=1.0)  # std1
            nc.vector.tensor_mul(tmp_c[:], tmp_b[:], cvec[:, :, CV_B1])       # B1*std1
            nc.vector.tensor_sub(negt1[:], tmp_c[:], tmp_a[:])                # B1*std1 - m1

            # =============== conv pass helper ===============
            def conv_pass(widx, wt, stats, prep, u_pool, a_col):
                """One binary conv over all images.

                prep(n, s, part) emits the sign-write of image n into s:
                part 0 = image rows 0..31 (enough for bands 0-2), part 1 =
                rows 32..55 (handled by prep itself; called with both parts
                back to back except for image 0 where part 0 is early).

                stats col j = (k*2+cc)*28 + n*7 + q  (k: 0=sum u, 1=sum u^2;
                q = chunk/half index, unused cols stay zero).
                Returns list of u tiles [128, 2, H, W] fp16 (y-units).
                """
                u_tiles = []
                prep(0, s_tiles[0], 0)
                prep(0, s_tiles[0], 1)
                for n in range(BL):
                    s = s_tiles[n % 2]
                    if n + 1 < BL:
                        prep(n + 1, s_tiles[(n + 1) % 2], 0)
                        prep(n + 1, s_tiles[(n + 1) % 2], 1)
                    ut = u_pool.tile([128, 2, H, W], F16, tag=f"u{widx}", name=f"u{widx}_{n}")
                    u_tiles.append(ut)
                    last = n == BL - 1
                    for cc in range(2):
                        for ci, (b0, nb) in enumerate(CHUNKS):
                            pt = psum_pool.tile(
                                [128, nb, BANK], F32,
                                tag="pt2" if nb == 2 else "pt1",
                                bufs=3 if nb == 2 else 2,
                                name=f"pt{widx}_{n}_{cc}_{ci}",
                            )
                            for k in range(nb):
                                b = b0 + k
                                po = pt[:, k, 0:NFREE]
                                for o, (dh, dw) in enumerate(OFFS):
                                    start = (b * BAND + dh) * PW + dw
                                    nc.tensor.matmul(
                                        po,
                                        wt[:, :, (o * 2 + cc) * 128:(o * 2 + cc + 1) * 128],
                                        s[:, :, start:start + NFREE],
                                        start=(o == 0), stop=(o == 8),
                                        perf_mode=DR,
                                    )
                            # copy psum chunk -> u rows (strided 4D read);
                            # cc0 on ACT, cc1 on DVE to balance engine load
                            pv = pt[:, :, 0:NFREE].rearrange(
                                "p k (r w) -> p k r w", w=PW)[:, :, :, 0:W]
                            us = ut[:, cc, b0 * BAND:(b0 + nb) * BAND, :]
                            ud = us.rearrange("p (k r) w -> p k r w", k=nb)
                            if cc == 0:
                                nc.scalar.activation(
                                    ud, pv, AF.Identity, bias=0.0, scale=1.0,
                                )
                            else:
                                nc.vector.tensor_scalar(
                                    ud, pv, 1.0, None, op0=OP.mult,
                                )
                            # u = max(y, a*y) in place per chunk, accum Σu
                            j0 = (0 * 2 + cc) * 28 + n * NBAND + ci
                            nc.vector.scalar_tensor_tensor(
                                us, us, cvec[:, cc, a_col:a_col + 1], us,
                                op0=OP.mult, op1=OP.max,
                                accum_out=stats[:, j0:j0 + 1],
                            )

                        # sum(u^2) per (cc, n): ACT when it has slack,
                        # GpSimd+DVE otherwise; last image on ACT halves so
                        # the stats tail after the final band stays short
                        j1 = (1 * 2 + cc) * 28 + n * NBAND
                        uf = ut[:, cc, :, :].rearrange("p h w -> p (h w)")
                        act_sq = (cc == 0 and n != 1) or last
                        if act_sq and not last:
                            nc.scalar.activation(
                                sqs[:], uf, AF.Square, bias=0.0, scale=1.0,
                                accum_out=stats[:, j1:j1 + 1],
                            )
                        elif act_sq:
                            for h in range(2):
                                nc.scalar.activation(
                                    sqs[:, 0:HHW], uf[:, h * HHW:(h + 1) * HHW],
                                    AF.Square, bias=0.0, scale=1.0,
                                    accum_out=stats[:, j1 + h:j1 + h + 1],
                                )
                        else:
                            for h in range(2):
                                uh = uf[:, h * HHW:(h + 1) * HHW]
                                nc.gpsimd.tensor_tensor(sqb[:], uh, uh, op=OP.mult)
                                nc.vector.tensor_scalar(
                                    sqb[:], sqb[:], 1.0, 0.0,
                                    op0=OP.mult, op1=OP.add,
                                    accum_out=stats[:, j1 + h:j1 + h + 1],
                                )
                return u_tiles

            # =============== conv1 ===============
            def prep1(n, s, part):
                r0, r1_ = (0, 32) if part == 0 else (32, 56)
                for g in range(2):
                    nc.scalar.activation(
                        s_plane(s, g)[:, 1 + r0:1 + r1_, 1:57],
                        x1s[n][g].rearrange("p (h w) -> p h w", w=W)[:, r0:r1_, :],
                        AF.Sign, bias=negt1[:, g:g + 1], scale=1.0,
                    )

            u1 = conv_pass(0, w1t, st2, prep1, u1_pool, CV_A1)
            es_xs.close()  # f32 x stream fully consumed

            # reduce st2 -> r2, AllGather -> g2
            r2 = cpool.tile([128, 4], F32, tag="r2")
            nc.vector.reduce_sum(
                r2[:].rearrange("p (a b) -> p a b", b=1),
                st2[:].rearrange("p (kc t) -> p kc t", t=28),
                axis=mybir.AxisListType.X,
            )
            ar2_i = dram_pool.tile([128, 4], F32, tag="ar2_i")
            ar2_o = dram_pool.tile([NCORES, 128, 4], F32, tag="ar2_o", addr_space="Shared")
            nc.sync.dma_start(ar2_i[:], r2[:])
            nc.gpsimd.collective_compute(
                "AllGather", OP.bypass, replica_groups=[list(range(NCORES))],
                ins=[ar2_i[:].opt()], outs=[ar2_o[:].opt()],
            )
            gth2 = cpool.tile([128, 4, NCORES], F32, tag="gth2")
            nc.scalar.dma_start(gth2[:], ar2_o[:].rearrange("r p k -> p k r"))
            nc.vector.reduce_sum(
                g2[:].rearrange("p (a b) -> p a b", b=1), gth2[:],
                axis=mybir.AxisListType.X,
            )

            # ---- BN2 threshold in u1 units: negth2 = (B2*std2 - m2)/sf1 ----
            g2v = g2[:].rearrange("p (k c) -> p k c", k=2)
            nc.vector.tensor_mul(tmp_a[:], g2v[:, 0], cvec[:, :, CV_SF1N])    # m2
            nc.vector.tensor_mul(tmp_b[:], g2v[:, 1], cvec[:, :, CV_SF1SQN])  # E[p1^2]
            nc.vector.scalar_tensor_tensor(
                tmp_c[:], tmp_a[:], -1.0, tmp_a[:], op0=OP.mult, op1=OP.mult,
            )
            nc.vector.tensor_add(tmp_c[:], tmp_c[:], tmp_b[:])                # v2
            nc.scalar.activation(tmp_b[:], tmp_c[:], AF.Sqrt, bias=cvec[:, 0, CV_EPS:CV_EPS + 1], scale=1.0)  # std2
            nc.vector.tensor_mul(tmp_c[:], tmp_b[:], cvec[:, :, CV_B2])       # B2*std2
            nc.vector.tensor_sub(tmp_a[:], tmp_a[:], tmp_c[:])                # t2 = m2 - B2*std2
            nc.vector.tensor_mul(tmp_a[:], tmp_a[:], cvec[:, :, CV_ISF1])     # theta (u units)
            nc.vector.tensor_scalar_mul(negth2[:], tmp_a[:], -1.0)

            # =============== conv2 ===============
            def prep2(n, s, part):
                r0, r1_ = (0, 32) if part == 0 else (32, 56)
                for g in range(2):
                    nc.scalar.activation(
                        s_plane(s, g)[:, 1 + r0:1 + r1_, 1:57],
                        u1[n][:, g, r0:r1_, :],
                        AF.Sign, bias=negth2[:, g:g + 1], scale=1.0,
                    )

            u2 = conv_pass(1, w2t, st3, prep2, u2_pool, CV_A2)

            # u1 fully consumed by prep2; release its pool
            es_u1.close()

            # reduce st3 -> r3, AllGather -> g3t
            r3 = cpool.tile([128, 4], F32, tag="r3")
            nc.vector.reduce_sum(
                r3[:].rearrange("p (a b) -> p a b", b=1),
                st3[:].rearrange("p (kc t) -> p kc t", t=28),
                axis=mybir.AxisListType.X,
            )
            ar3_i = dram_pool.tile([128, 4], F32, tag="ar3_i")
            ar3_o = dram_pool.tile([NCORES, 128, 4], F32, tag="ar3_o", addr_space="Shared")
            nc.sync.dma_start(ar3_i[:], r3[:])
            nc.gpsimd.collective_compute(
                "AllGather", OP.bypass, replica_groups=[list(range(NCORES))],
                ins=[ar3_i[:].opt()], outs=[ar3_o[:].opt()],
            )
            gth3 = cpool.tile([128, 4, NCORES], F32, tag="gth3")
            nc.scalar.dma_start(gth3[:], ar3_o[:].rearrange("r p k -> p k r"))
            nc.vector.reduce_sum(
                g3t[:].rearrange("p (a b) -> p a b", b=1), gth3[:],
                axis=mybir.AxisListType.X,
            )

            # ---- BN3 affine: K = g3*sf2*rstd3, D = b3 - m3*g3*rstd3 ----
            g3v = g3t[:].rearrange("p (k c) -> p k c", k=2)
            nc.vector.tensor_mul(tmp_a[:], g3v[:, 0], cvec[:, :, CV_SF2N])    # m3
            nc.vector.tensor_mul(tmp_b[:], g3v[:, 1], cvec[:, :, CV_SF2SQN])  # E[p2^2]
            nc.vector.scalar_tensor_tensor(
                tmp_c[:], tmp_a[:], -1.0, tmp_a[:], op0=OP.mult, op1=OP.mult,
            )
            nc.vector.tensor_add(tmp_c[:], tmp_c[:], tmp_b[:])                # v3
            nc.scalar.activation(tmp_b[:], tmp_c[:], AF.Sqrt, bias=cvec[:, 0, CV_EPS:CV_EPS + 1], scale=1.0)  # std3
            nc.vector.reciprocal(tmp_c[:], tmp_b[:])                          # rstd3
            nc.vector.tensor_mul(kvec[:], tmp_c[:], cvec[:, :, CV_G3SF2])     # K
            nc.vector.tensor_mul(tmp_a[:], tmp_a[:], cvec[:, :, CV_G3])       # m3*g3
            nc.vector.tensor_mul(tmp_a[:], tmp_a[:], tmp_c[:])                # m3*g3*rstd3
            nc.vector.tensor_sub(dvec[:], cvec[:, :, CV_B3V], tmp_a[:])       # D

            # ====== phase 3: out = prelu(K*u2 + D + x, a3), fp16 out ======
            with tc.tile_pool(name="ph3", bufs=2) as p3pool:
                for n in range(BL):
                    th = p3pool.tile([128, 2, HW], F16, tag="th", name=f"th_{n}")
                    ot = p3pool.tile([128, 2, HW], F16, tag="ot", name=f"ot_{n}")
                    for g in range(2):
                        nc.scalar.activation(
                            th[:, g, :],
                            u2[n][:, g, :, :].rearrange("p h w -> p (h w)"),
                            AF.Identity,
                            bias=dvec[:, g:g + 1], scale=kvec[:, g:g + 1],
                        )
                    for g in range(2):
                        # w = th + x on GpSimd (idle in phase 3)
                        nc.gpsimd.tensor_tensor(
                            th[:, g, :], th[:, g, :], x16s[n][:, g, :], op=OP.add,
                        )
                        # aw = a3*w (4x ts), out = max(w, aw) (2x tt)
                        nc.vector.tensor_scalar(
                            ot[:, g, :], th[:, g, :],
                            cvec[:, g, CV_A3:CV_A3 + 1], None, op0=OP.mult,
                        )
                        nc.vector.tensor_tensor(
                            ot[:, g, :], ot[:, g, :], th[:, g, :], op=OP.max,
                        )
                    nc.sync.dma_start(out_view(n), ot[:])

    nc.compile()
    return nc


def _host_prep(inputs):
    x = np.ascontiguousarray(np.asarray(inputs["x"], dtype=np.float32))
    w1 = np.asarray(inputs["w1"], dtype=np.float32)
    w2 = np.asarray(inputs["w2"], dtype=np.float32)

    def wprep(w):
        ws = np.sign(w).astype(np.float32)  # [co, ci, kh, kw]
        sf = np.abs(w).mean(axis=(1, 2, 3)).astype(np.float32)  # [256]
        arr = np.empty((128, 2, 18, 128), dtype=np.float32)
        for o, (dh, dw) in enumerate(OFFS):
            for cc in range(2):
                t = ws[cc * 128:(cc + 1) * 128, :, dh, dw]  # [m, ci]
                # arr[p, g, blk, m] = t[m, g*128 + p]
                arr[:, :, o * 2 + cc, :] = t.T.reshape(2, 128, 128).transpose(1, 0, 2)
        return arr.reshape(128, 2, 18 * 128).astype(F8NP), sf

    w1s, sf1 = wprep(w1)
    w2s, sf2 = wprep(w2)

    def vec(v):
        return np.asarray(v, dtype=np.float32).reshape(2, 128).T  # [p, g]

    g1v, b1v = inputs["g1"], inputs["b1"]
    g2v, b2v = inputs["g2"], inputs["b2"]
    g3v, b3v = inputs["g3"], inputs["b3"]
    a1, a2, a3 = inputs["a1"], inputs["a2"], inputs["a3"]

    cvec = np.zeros((128, 2, CV_NCOLS), dtype=np.float32)
    cvec[:, :, CV_B1] = vec(np.asarray(b1v) / np.asarray(g1v))
    cvec[:, :, CV_A1] = vec(np.asarray(a1))
    cvec[:, :, CV_SF1N] = vec(sf1 / NTOT_)
    cvec[:, :, CV_SF1SQN] = vec(sf1 * sf1 / NTOT_)
    cvec[:, :, CV_B2] = vec(np.asarray(b2v) / np.asarray(g2v))
    cvec[:, :, CV_ISF1] = vec(1.0 / sf1)
    cvec[:, :, CV_A2] = vec(np.asarray(a2))
    cvec[:, :, CV_SF2N] = vec(sf2 / NTOT_)
    cvec[:, :, CV_SF2SQN] = vec(sf2 * sf2 / NTOT_)
    cvec[:, :, CV_G3SF2] = vec(np.asarray(g3v) * sf2)
    cvec[:, :, CV_G3] = vec(np.asarray(g3v))
    cvec[:, :, CV_B3V] = vec(np.asarray(b3v))
    cvec[:, :, CV_A3] = vec(np.asarray(a3))
    cvec[:, :, CV_EPS] = EPS

    return x, w1s, w2s, cvec


def run(inputs, trace=False):
    x, w1s, w2s, cvec = _host_prep(inputs)
    if "nc" not in _CACHE:
        _CACHE["nc"] = _build()
    nc = _CACHE["nc"]
    in_maps = [
        {"x": x[i * BL:(i + 1) * BL], "w1s": w1s, "w2s": w2s, "cvec": cvec}
        for i in range(NCORES)
    ]
    res = run_bass_kernel_spmd(nc, in_maps, list(range(NCORES)), trace=trace)
    out = np.concatenate([res.results[i]["out"] for i in range(NCORES)], axis=0)
    return out.astype(np.float32), res


def kernel(**inputs):
    out, _ = run(inputs, trace=False)
    return out


if __name__ == "__main__":
    # build-only check
    _build()
    print("BUILD OK")


# revision 28
# speedup vs baseline: 1.0626x; 1.0626x over previous
"""XNOR-Net BasicBlock (BN-sign-conv x2 + residual, training-mode BN) on 8 TRN2 cores.

Strategy (data-parallel on batch, 4 images/core):
  phase0: x streamed fp16 (gpsimd cast DMA, kept for the phase-3 residual);
          per-channel sum via DVE ts-accum, sumsq via ACT Square / DVE
          tensor_tensor_reduce -> AllGather (BN1 stats)
  conv1 : x re-streamed f32 (overlaps AR1) for exact s1 = sign(x - t1) fp8;
          3x3 conv as 9 DoubleRow fp8 matmuls per 8-row band into 4-band
          PSUM tiles; epilogue per 4/3-band chunk: DVE copy psum->u (fp16),
          in-place DVE stt u = max(y, a*y) with accum -> sum(u);
          sum(u^2): cc0 via ACT Square+accum, cc1 via GpSimd square +
          DVE ts-accum  -> AllGather (BN2 stats)
  conv2 : s2 = sign(u1 - theta2), same -> AllGather (BN3 stats)
  phase3: out = prelu(K*u2 + D + x, a3) in fp16 (host casts to f32)
"""

import sys

sys.path.insert(0, "/opt/trn_rl_repo")

import contextlib

import numpy as np

import concourse.bacc as bacc
import concourse.mybir as mybir
import concourse.tile as tile
from concourse.bass_utils import run_bass_kernel_spmd

F32 = mybir.dt.float32
F16 = mybir.dt.float16
BF16 = mybir.dt.bfloat16
F8 = mybir.dt.float8e4
F8NP = mybir.dt.np(F8)

AF = mybir.ActivationFunctionType
OP = mybir.AluOpType
DR = mybir.MatmulPerfMode.DoubleRow

NCORES = 8
B, C, H, W = 32, 256, 56, 56
BL = B // NCORES          # images per core
HW = H * W                # 3136
HHW = HW // 2             # 1568 half plane
PW = W + 2                # 58 padded width
PLANE = PW * PW           # 3364 padded plane (58 rows x 58 cols)
PLANE_PAD = 3392          # plane stride, %16 == 0
BAND = 8                  # output rows per matmul
NBAND = H // BAND         # 7
NFREE = BAND * PW         # 464 psum free size per band
BANK = 512                # psum band stride (one 2KB bank)
NTOT = B * HW             # BN count (N*H*W over full batch)
EPS = 1e-5
OFFS = [(dh, dw) for dh in range(3) for dw in range(3)]
CHUNKS = ((0, 2), (2, 2), (4, 2), (6, 1))  # (first band, nbands) psum chunks

NTOT_ = float(NTOT)

# cvec column indices
CV_B1, CV_A1, CV_SF1N, CV_SF1SQN, CV_B2, CV_ISF1, CV_A2, \
    CV_SF2N, CV_SF2SQN, CV_G3SF2, CV_G3, CV_B3V, CV_A3, CV_EPS = range(14)
CV_NCOLS = 14

_CACHE = {}


def _build():
    nc = bacc.Bacc(num_devices=NCORES)
    x_d = nc.declare_dram_parameter("x", [BL, C, H, W], F32, isOutput=False)
    w1_d = nc.declare_dram_parameter("w1s", [128, 2, 18 * 128], F8, isOutput=False)
    w2_d = nc.declare_dram_parameter("w2s", [128, 2, 18 * 128], F8, isOutput=False)
    cv_d = nc.declare_dram_parameter("cvec", [128, 2, CV_NCOLS], F32, isOutput=False)
    out_d = nc.declare_dram_parameter("out", [BL, C, H, W], F16, isOutput=True)

    # DRAM-side views: channel c -> (g = c // 128, p = c % 128)
    def x_view(n):
        return x_d[n].rearrange("(g p) h w -> p g (h w)", p=128)

    def out_view(n):
        return out_d[n].rearrange("(g p) h w -> p g (h w)", p=128)

    with tile.TileContext(nc, num_cores=NCORES, pool_alloc_mode="queue") as tc:
        es_u1 = contextlib.ExitStack()
        es_u2 = contextlib.ExitStack()
        es_xs = contextlib.ExitStack()
        with tc.tile_pool(name="consts", bufs=1) as cpool, \
                tc.tile_pool(name="weights", bufs=1) as wpool, \
                tc.tile_pool(name="spool", bufs=1) as spool, \
                tc.tile_pool(name="sqscr", bufs=1) as sqpool, \
                tc.tile_pool(name="psum", bufs=1, space="PSUM") as psum_pool, \
                tc.tile_pool(name="dram", bufs=1, space="DRAM") as dram_pool, \
                es_u2:

            # ---- persistent small tiles ----
            cvec = cpool.tile([128, 2, CV_NCOLS], F32, tag="cvec")
            st1 = cpool.tile([128, 32], F32, tag="st1")
            st2 = cpool.tile([128, 112], F32, tag="st2")
            st3 = cpool.tile([128, 112], F32, tag="st3")
            g1 = cpool.tile([128, 4], F32, tag="g1")
            g2 = cpool.tile([128, 4], F32, tag="g2")
            g3t = cpool.tile([128, 4], F32, tag="g3t")
            negt1 = cpool.tile([128, 2], F32, tag="negt1")
            negth2 = cpool.tile([128, 2], F32, tag="negth2")
            kvec = cpool.tile([128, 2], F32, tag="kvec")
            dvec = cpool.tile([128, 2], F32, tag="dvec")
            tmp_a = cpool.tile([128, 2], F32, tag="tmp_a")
            tmp_b = cpool.tile([128, 2], F32, tag="tmp_b")
            tmp_c = cpool.tile([128, 2], F32, tag="tmp_c")

            w1t = wpool.tile([128, 2, 18 * 128], F8, tag="w1t")
            w2t = wpool.tile([128, 2, 18 * 128], F8, tag="w2t")

            s_tiles = [
                spool.tile([128, 2, PLANE_PAD], F8, tag="sa", name="sa"),
                spool.tile([128, 2, PLANE_PAD], F8, tag="sb", name="sb"),
            ]

            # shared square scratch (garbage out): sqs for ACT squares,
            # sqb (half-size) for GpSimd/DVE squares — separate tags so the
            # two engines' square chains don't serialize on WAW deps
            sqs = sqpool.tile([128, HW], F16, tag="sqs")
            sqb = sqpool.tile([128, HHW], BF16, tag="sqb")

            # persistent fp16 x (phase0 stats source + phase3 residual), u2
            x16_pool = es_u2.enter_context(tc.tile_pool(name="x16", bufs=BL))
            u2_pool = es_u2.enter_context(tc.tile_pool(name="u2", bufs=BL))
            u1_pool = es_u1.enter_context(tc.tile_pool(name="u1", bufs=BL))

            # =============== phase 0: fp16 x stream + stats ===============
            # st1 col j = (k*2 + g)*8 + n*2 + h  (k: 0=sum, 1=sumsq; h=half)
            # cast DMAs first on the gpsimd queue so their descriptor gen
            # isn't stuck behind the big s-tile memsets
            # stats-tile memsets first on the pool queue: the stats accums
            # wait on these (WAW), so they must precede casts and s-memsets
            nc.gpsimd.memset(st1[:], 0.0)
            nc.gpsimd.memset(st2[:], 0.0)
            nc.gpsimd.memset(st3[:], 0.0)
            x16s = []
            for n in range(BL):
                x16 = x16_pool.tile([128, 2, HW], F16, tag="x16", name=f"x16_{n}")
                x16s.append(x16)
                for g in range(2):
                    # cast f32 -> fp16 in flight (gpsimd DGE)
                    nc.gpsimd.dma_start(x16[:, g, :], x_view(n)[:, g, :])

            nc.sync.dma_start(cvec[:], cv_d[:])
            nc.sync.dma_start(w1t[:], w1_d[:])
            nc.sync.dma_start(w2t[:], w2_d[:])
            for s in s_tiles:
                nc.gpsimd.memset(s[:], 0.0)

            def s_plane(s, g):
                return s[:, g, 0:PLANE].rearrange("p (r w) -> p r w", w=PW)

            for n in range(BL):
                x16 = x16s[n]
                for g in range(2):
                    # sum(x) via in-place ts-accum (4x DVE mode)
                    nc.vector.tensor_scalar(
                        x16[:, g, :], x16[:, g, :], 1.0, 0.0,
                        op0=OP.mult, op1=OP.add,
                        accum_out=st1[:, (0 * 2 + g) * 8 + n * 2:(0 * 2 + g) * 8 + n * 2 + 1],
                    )
                    jq = (1 * 2 + g) * 8 + n * 2
                    if n == 1:
                        # pool square halves + DVE ts-accum: takes two of the
                        # eight squares off the serial ACT chain
                        for h in range(2):
                            xh = x16[:, g, h * HHW:(h + 1) * HHW]
                            nc.gpsimd.tensor_tensor(sqb[:], xh, xh, op=OP.mult)
                            nc.vector.tensor_scalar(
                                sqb[:], sqb[:], 1.0, 0.0, op0=OP.mult, op1=OP.add,
                                accum_out=st1[:, jq + h:jq + h + 1],
                            )
                    elif n < 3:
                        # ACT square, exact f32 accum
                        nc.scalar.activation(
                            sqs[:], x16[:, g, :], AF.Square, bias=0.0, scale=1.0,
                            accum_out=st1[:, jq:jq + 1],
                        )
                    else:
                        # last image: ACT halves keep the stats tail short
                        for h in range(2):
                            xh = x16[:, g, h * HHW:(h + 1) * HHW]
                            nc.scalar.activation(
                                sqs[:, 0:HHW], xh, AF.Square, bias=0.0,
                                scale=1.0, accum_out=st1[:, jq + h:jq + h + 1],
                            )

            # reduce st1 [128, (k g) 8] -> r1 [128, 4], AllGather -> g1
            r1 = cpool.tile([128, 4], F32, tag="r1")
            nc.vector.reduce_sum(
                r1[:].rearrange("p (a b) -> p a b", b=1),
                st1[:].rearrange("p (kg t) -> p kg t", t=2 * BL),
                axis=mybir.AxisListType.X,
            )
            ar1_i = dram_pool.tile([128, 4], F32, tag="ar1_i")
            ar1_o = dram_pool.tile([NCORES, 128, 4], F32, tag="ar1_o", addr_space="Shared")
            nc.sync.dma_start(ar1_i[:], r1[:])
            nc.gpsimd.collective_compute(
                "AllGather", OP.bypass, replica_groups=[list(range(NCORES))],
                ins=[ar1_i[:].opt()], outs=[ar1_o[:].opt()],
            )
            # gth read on the ACT hwdge queue so SP can run the conv1 x
            # re-stream during the collective
            gth1 = cpool.tile([128, 4, NCORES], F32, tag="gth1")
            nc.scalar.dma_start(gth1[:], ar1_o[:].rearrange("r p k -> p k r"))
            nc.vector.reduce_sum(
                g1[:].rearrange("p (a b) -> p a b", b=1), gth1[:],
                axis=mybir.AxisListType.X,
            )

            # conv1 f32 x re-stream (per half-image); transfers overlap AR1
            xs_pool = es_xs.enter_context(tc.tile_pool(name="xs", bufs=2))
            x1s = []
            for n in range(BL):
                halves = []
                for g in range(2):
                    xt = xs_pool.tile([128, HW], F32, tag="xt", name=f"x1_{n}_{g}")
                    nc.sync.dma_start(xt[:], x_view(n)[:, g, :])
                    halves.append(xt)
                x1s.append(halves)

            # ---- BN1 threshold: negt1 = B1*std1 - m1 ----
            g1v = g1[:].rearrange("p (k g) -> p k g", k=2)
            nc.vector.tensor_scalar_mul(tmp_a[:], g1v[:, 0], 1.0 / NTOT_)     # m1
            nc.vector.tensor_scalar_mul(tmp_b[:], g1v[:, 1], 1.0 / NTOT_)     # E[x^2]
            nc.vector.scalar_tensor_tensor(                                   # -m^2
                tmp_c[:], tmp_a[:], -1.0, tmp_a[:], op0=OP.mult, op1=OP.mult,
            )
            nc.vector.tensor_add(tmp_c[:], tmp_c[:], tmp_b[:])                # v1
            nc.scalar.activation(tmp_b[:], tmp_c[:], AF.Sqrt, bias=cvec[:, 0, CV_EPS:CV_EPS + 1], scale=1.0)  # std1
            nc.vector.tensor_mul(tmp_c[:], tmp_b[:], cvec[:, :, CV_B1])       # B1*std1
            nc.vector.tensor_sub(negt1[:], tmp_c[:], tmp_a[:])                # B1*std1 - m1

            # =============== conv pass helper ===============
            def conv_pass(widx, wt, stats, prep, u_pool, a_col):
                """One binary conv over all images.

                prep(n, s, part) emits the sign-write of image n into s:
                part 0 = image rows 0..31 (enough for bands 0-2), part 1 =
                rows 32..55 (handled by prep itself; called with both parts
                back to back except for image 0 where part 0 is early).

                stats col j = (k*2+cc)*28 + n*7 + q  (k: 0=sum u, 1=sum u^2;
                q = chunk/half index, unused cols stay zero).
                Returns list of u tiles [128, 2, H, W] fp16 (y-units).
                """
                u_tiles = []
                prep(0, s_tiles[0], 0)
                prep(0, s_tiles[0], 1)
                for n in range(BL):
                    s = s_tiles[n % 2]
                    ut = u_pool.tile([128, 2, H, W], F16, tag=f"u{widx}", name=f"u{widx}_{n}")
                    u_tiles.append(ut)
                    last = n == BL - 1
                    for cc in range(2):
                        for ci, (b0, nb) in enumerate(CHUNKS):
                            pt = psum_pool.tile(
                                [128, nb, BANK], F32,
                                tag="pt2" if nb == 2 else "pt1",
                                bufs=3 if nb == 2 else 2,
                                name=f"pt{widx}_{n}_{cc}_{ci}",
                            )
                            for k in range(nb):
                                b = b0 + k
                                po = pt[:, k, 0:NFREE]
                                for o, (dh, dw) in enumerate(OFFS):
                                    start = (b * BAND + dh) * PW + dw
                                    nc.tensor.matmul(
                                        po,
                                        wt[:, :, (o * 2 + cc) * 128:(o * 2 + cc + 1) * 128],
                                        s[:, :, start:start + NFREE],
                                        start=(o == 0), stop=(o == 8),
                                        perf_mode=DR,
                                    )
                            # copy psum chunk -> u rows (strided 4D read);
                            # cc0 on ACT, cc1 on DVE to balance engine load.
                            # The copy is the psum bank's only reader, so
                            # banks recycle without waiting on the prelu.
                            pv = pt[:, :, 0:NFREE].rearrange(
                                "p k (r w) -> p k r w", w=PW)[:, :, :, 0:W]
                            us = ut[:, cc, b0 * BAND:(b0 + nb) * BAND, :]
                            ud = us.rearrange("p (k r) w -> p k r w", k=nb)
                            if cc == 0:
                                nc.scalar.activation(
                                    ud, pv, AF.Identity, bias=0.0, scale=1.0,
                                )
                            else:
                                nc.vector.tensor_scalar(
                                    ud, pv, 1.0, None, op0=OP.mult,
                                )
                            # u = max(y, a*y) in place per chunk, accum Σu
                            j0 = (0 * 2 + cc) * 28 + n * NBAND + ci
                            nc.vector.scalar_tensor_tensor(
                                us, us, cvec[:, cc, a_col:a_col + 1], us,
                                op0=OP.mult, op1=OP.max,
                                accum_out=stats[:, j0:j0 + 1],
                            )
                        # next image's sign-prep part between the cc phases:
                        # during cc1 the ACT queue is otherwise free
                        if n + 1 < BL:
                            prep(n + 1, s_tiles[(n + 1) % 2], cc)
                    # sum(u^2) per (cc, n) at image end: ACT when it has
                    # slack, GpSimd+DVE otherwise; last image on ACT halves
                    # so the stats tail after the final band stays short
                    for cc in range(2):
                        j1 = (1 * 2 + cc) * 28 + n * NBAND
                        uf = ut[:, cc, :, :].rearrange("p h w -> p (h w)")
                        act_sq = (cc == 0 and n != 1) or last
                        if act_sq and not last:
                            nc.scalar.activation(
                                sqs[:], uf, AF.Square, bias=0.0, scale=1.0,
                                accum_out=stats[:, j1:j1 + 1],
                            )
                        elif act_sq:
                            for h in range(2):
                                nc.scalar.activation(
                                    sqs[:, 0:HHW], uf[:, h * HHW:(h + 1) * HHW],
                                    AF.Square, bias=0.0, scale=1.0,
                                    accum_out=stats[:, j1 + h:j1 + h + 1],
                                )
                        else:
                            for h in range(2):
                                uh = uf[:, h * HHW:(h + 1) * HHW]
                                nc.gpsimd.tensor_tensor(sqb[:], uh, uh, op=OP.mult)
                                nc.vector.tensor_scalar(
                                    sqb[:], sqb[:], 1.0, 0.0,
                                    op0=OP.mult, op1=OP.add,
                                    accum_out=stats[:, j1 + h:j1 + h + 1],
                                )
                return u_tiles

            # =============== conv1 ===============
            def prep1(n, s, part):
                r0, r1_ = (0, 32) if part == 0 else (32, 56)
                for g in range(2):
                    nc.scalar.activation(
                        s_plane(s, g)[:, 1 + r0:1 + r1_, 1:57],
                        x1s[n][g].rearrange("p (h w) -> p h w", w=W)[:, r0:r1_, :],
                        AF.Sign, bias=negt1[:, g:g + 1], scale=1.0,
                    )

            u1 = conv_pass(0, w1t, st2, prep1, u1_pool, CV_A1)
            es_xs.close()  # f32 x stream fully consumed

            # reduce st2 -> r2, AllGather -> g2
            r2 = cpool.tile([128, 4], F32, tag="r2")
            nc.vector.reduce_sum(
                r2[:].rearrange("p (a b) -> p a b", b=1),
                st2[:].rearrange("p (kc t) -> p kc t", t=28),
                axis=mybir.AxisListType.X,
            )
            ar2_i = dram_pool.tile([128, 4], F32, tag="ar2_i")
            ar2_o = dram_pool.tile([NCORES, 128, 4], F32, tag="ar2_o", addr_space="Shared")
            nc.sync.dma_start(ar2_i[:], r2[:])
            nc.gpsimd.collective_compute(
                "AllGather", OP.bypass, replica_groups=[list(range(NCORES))],
                ins=[ar2_i[:].opt()], outs=[ar2_o[:].opt()],
            )
            gth2 = cpool.tile([128, 4, NCORES], F32, tag="gth2")
            nc.scalar.dma_start(gth2[:], ar2_o[:].rearrange("r p k -> p k r"))
            nc.vector.reduce_sum(
                g2[:].rearrange("p (a b) -> p a b", b=1), gth2[:],
                axis=mybir.AxisListType.X,
            )

            # ---- BN2 threshold in u1 units: negth2 = (B2*std2 - m2)/sf1 ----
            g2v = g2[:].rearrange("p (k c) -> p k c", k=2)
            nc.vector.tensor_mul(tmp_a[:], g2v[:, 0], cvec[:, :, CV_SF1N])    # m2
            nc.vector.tensor_mul(tmp_b[:], g2v[:, 1], cvec[:, :, CV_SF1SQN])  # E[p1^2]
            nc.vector.scalar_tensor_tensor(
                tmp_c[:], tmp_a[:], -1.0, tmp_a[:], op0=OP.mult, op1=OP.mult,
            )
            nc.vector.tensor_add(tmp_c[:], tmp_c[:], tmp_b[:])                # v2
            nc.scalar.activation(tmp_b[:], tmp_c[:], AF.Sqrt, bias=cvec[:, 0, CV_EPS:CV_EPS + 1], scale=1.0)  # std2
            nc.vector.tensor_mul(tmp_c[:], tmp_b[:], cvec[:, :, CV_B2])       # B2*std2
            nc.vector.tensor_sub(tmp_a[:], tmp_a[:], tmp_c[:])                # t2 = m2 - B2*std2
            nc.vector.tensor_mul(tmp_a[:], tmp_a[:], cvec[:, :, CV_ISF1])     # theta (u units)
            nc.vector.tensor_scalar_mul(negth2[:], tmp_a[:], -1.0)

            # =============== conv2 ===============
            def prep2(n, s, part):
                r0, r1_ = (0, 32) if part == 0 else (32, 56)
                for g in range(2):
                    nc.scalar.activation(
                        s_plane(s, g)[:, 1 + r0:1 + r1_, 1:57],
                        u1[n][:, g, r0:r1_, :],
                        AF.Sign, bias=negth2[:, g:g + 1], scale=1.0,
                    )

            u2 = conv_pass(1, w2t, st3, prep2, u2_pool, CV_A2)

            # u1 fully consumed by prep2; release its pool
            es_u1.close()

            # reduce st3 -> r3, AllGather -> g3t
            r3 = cpool.tile([128, 4], F32, tag="r3")
            nc.vector.reduce_sum(
                r3[:].rearrange("p (a b) -> p a b", b=1),
                st3[:].rearrange("p (kc t) -> p kc t", t=28),
                axis=mybir.AxisListType.X,
            )
            ar3_i = dram_pool.tile([128, 4], F32, tag="ar3_i")
            ar3_o = dram_pool.tile([NCORES, 128, 4], F32, tag="ar3_o", addr_space="Shared")
            nc.sync.dma_start(ar3_i[:], r3[:])
            nc.gpsimd.collective_compute(
                "AllGather", OP.bypass, replica_groups=[list(range(NCORES))],
                ins=[ar3_i[:].opt()], outs=[ar3_o[:].opt()],
            )
            gth3 = cpool.tile([128, 4, NCORES], F32, tag="gth3")
            nc.scalar.dma_start(gth3[:], ar3_o[:].rearrange("r p k -> p k r"))
            nc.vector.reduce_sum(
                g3t[:].rearrange("p (a b) -> p a b", b=1), gth3[:],
                axis=mybir.AxisListType.X,
            )

            # ---- BN3 affine: K = g3*sf2*rstd3, D = b3 - m3*g3*rstd3 ----
            g3v = g3t[:].rearrange("p (k c) -> p k c", k=2)
            nc.vector.tensor_mul(tmp_a[:], g3v[:, 0], cvec[:, :, CV_SF2N])    # m3
            nc.vector.tensor_mul(tmp_b[:], g3v[:, 1], cvec[:, :, CV_SF2SQN])  # E[p2^2]
            nc.vector.scalar_tensor_tensor(
                tmp_c[:], tmp_a[:], -1.0, tmp_a[:], op0=OP.mult, op1=OP.mult,
            )
            nc.vector.tensor_add(tmp_c[:], tmp_c[:], tmp_b[:])                # v3
            nc.scalar.activation(tmp_b[:], tmp_c[:], AF.Sqrt, bias=cvec[:, 0, CV_EPS:CV_EPS + 1], scale=1.0)  # std3
            nc.vector.reciprocal(tmp_c[:], tmp_b[:])                          # rstd3
            nc.vector.tensor_mul(kvec[:], tmp_c[:], cvec[:, :, CV_G3SF2])     # K
            nc.vector.tensor_mul(tmp_a[:], tmp_a[:], cvec[:, :, CV_G3])       # m3*g3
            nc.vector.tensor_mul(tmp_a[:], tmp_a[:], tmp_c[:])                # m3*g3*rstd3
            nc.vector.tensor_sub(dvec[:], cvec[:, :, CV_B3V], tmp_a[:])       # D

            # ====== phase 3: out = prelu(K*u2 + D + x, a3), fp16 out ======
            with tc.tile_pool(name="ph3", bufs=2) as p3pool:
                for n in range(BL):
                    th = p3pool.tile([128, 2, HW], F16, tag="th", name=f"th_{n}")
                    ot = p3pool.tile([128, 2, HW], F16, tag="ot", name=f"ot_{n}")
                    for g in range(2):
                        nc.scalar.activation(
                            th[:, g, :],
                            u2[n][:, g, :, :].rearrange("p h w -> p (h w)"),
                            AF.Identity,
                            bias=dvec[:, g:g + 1], scale=kvec[:, g:g + 1],
                        )
                    for g in range(2):
                        # w = th + x (2x tt), aw = a3*w (4x ts),
                        # out = max(w, aw) (2x tt) — per-g DVE pipeline
                        nc.vector.tensor_tensor(
                            th[:, g, :], th[:, g, :], x16s[n][:, g, :], op=OP.add,
                        )
                        nc.vector.tensor_scalar(
                            ot[:, g, :], th[:, g, :],
                            cvec[:, g, CV_A3:CV_A3 + 1], None, op0=OP.mult,
                        )
                        nc.vector.tensor_tensor(
                            ot[:, g, :], ot[:, g, :], th[:, g, :], op=OP.max,
                        )
                    nc.sync.dma_start(out_view(n), ot[:])

    nc.compile()
    return nc


def _host_prep(inputs):
    x = np.ascontiguousarray(np.asarray(inputs["x"], dtype=np.float32))
    w1 = np.asarray(inputs["w1"], dtype=np.float32)
    w2 = np.asarray(inputs["w2"], dtype=np.float32)

    def wprep(w):
        ws = np.sign(w).astype(np.float32)  # [co, ci, kh, kw]
        sf = np.abs(w).mean(axis=(1, 2, 3)).astype(np.float32)  # [256]
        arr = np.empty((128, 2, 18, 128), dtype=np.float32)
        for o, (dh, dw) in enumerate(OFFS):
            for cc in range(2):
                t = ws[cc * 128:(cc + 1) * 128, :, dh, dw]  # [m, ci]
                # arr[p, g, blk, m] = t[m, g*128 + p]
                arr[:, :, o * 2 + cc, :] = t.T.reshape(2, 128, 128).transpose(1, 0, 2)
        return arr.reshape(128, 2, 18 * 128).astype(F8NP), sf

    w1s, sf1 = wprep(w1)
    w2s, sf2 = wprep(w2)

    def vec(v):
        return np.asarray(v, dtype=np.float32).reshape(2, 128).T  # [p, g]

    g1v, b1v = inputs["g1"], inputs["b1"]
    g2v, b2v = inputs["g2"], inputs["b2"]
    g3v, b3v = inputs["g3"], inputs["b3"]
    a1, a2, a3 = inputs["a1"], inputs["a2"], inputs["a3"]

    cvec = np.zeros((128, 2, CV_NCOLS), dtype=np.float32)
    cvec[:, :, CV_B1] = vec(np.asarray(b1v) / np.asarray(g1v))
    cvec[:, :, CV_A1] = vec(np.asarray(a1))
    cvec[:, :, CV_SF1N] = vec(sf1 / NTOT_)
    cvec[:, :, CV_SF1SQN] = vec(sf1 * sf1 / NTOT_)
    cvec[:, :, CV_B2] = vec(np.asarray(b2v) / np.asarray(g2v))
    cvec[:, :, CV_ISF1] = vec(1.0 / sf1)
    cvec[:, :, CV_A2] = vec(np.asarray(a2))
    cvec[:, :, CV_SF2N] = vec(sf2 / NTOT_)
    cvec[:, :, CV_SF2SQN] = vec(sf2 * sf2 / NTOT_)
    cvec[:, :, CV_G3SF2] = vec(np.asarray(g3v) * sf2)
    cvec[:, :, CV_G3] = vec(np.asarray(g3v))
    cvec[:, :, CV_B3V] = vec(np.asarray(b3v))
    cvec[:, :, CV_A3] = vec(np.asarray(a3))
    cvec[:, :, CV_EPS] = EPS

    return x, w1s, w2s, cvec


def run(inputs, trace=False):
    x, w1s, w2s, cvec = _host_prep(inputs)
    if "nc" not in _CACHE:
        _CACHE["nc"] = _build()
    nc = _CACHE["nc"]
    in_maps = [
        {"x": x[i * BL:(i + 1) * BL], "w1s": w1s, "w2s": w2s, "cvec": cvec}
        for i in range(NCORES)
    ]
    res = run_bass_kernel_spmd(nc, in_maps, list(range(NCORES)), trace=trace)
    out = np.concatenate([res.results[i]["out"] for i in range(NCORES)], axis=0)
    return out.astype(np.float32), res


def kernel(**inputs):
    out, _ = run(inputs, trace=False)
    return out


if __name__ == "__main__":
    # build-only check
    _build()
    print("BUILD OK")


# revision 29
# speedup vs baseline: 1.0648x; 1.0020x over previous
"""XNOR-Net BasicBlock (BN-sign-conv x2 + residual, training-mode BN) on 8 TRN2 cores.

Strategy (data-parallel on batch, 4 images/core):
  phase0: x streamed fp16 (gpsimd cast DMA, kept for the phase-3 residual);
          per-channel sum via DVE ts-accum, sumsq via ACT Square / DVE
          tensor_tensor_reduce -> AllGather (BN1 stats)
  conv1 : x re-streamed f32 (overlaps AR1) for exact s1 = sign(x - t1) fp8;
          3x3 conv as 9 DoubleRow fp8 matmuls per 8-row band into 4-band
          PSUM tiles; epilogue per 4/3-band chunk: DVE copy psum->u (fp16),
          in-place DVE stt u = max(y, a*y) with accum -> sum(u);
          sum(u^2): cc0 via ACT Square+accum, cc1 via GpSimd square +
          DVE ts-accum  -> AllGather (BN2 stats)
  conv2 : s2 = sign(u1 - theta2), same -> AllGather (BN3 stats)
  phase3: out = prelu(K*u2 + D + x, a3) in fp16 (host casts to f32)
"""

import sys

sys.path.insert(0, "/opt/trn_rl_repo")

import contextlib

import numpy as np

import concourse.bacc as bacc
import concourse.mybir as mybir
import concourse.tile as tile
from concourse.bass_utils import run_bass_kernel_spmd

F32 = mybir.dt.float32
F16 = mybir.dt.float16
BF16 = mybir.dt.bfloat16
F8 = mybir.dt.float8e4
F8NP = mybir.dt.np(F8)

AF = mybir.ActivationFunctionType
OP = mybir.AluOpType
DR = mybir.MatmulPerfMode.DoubleRow

NCORES = 8
B, C, H, W = 32, 256, 56, 56
BL = B // NCORES          # images per core
HW = H * W                # 3136
HHW = HW // 2             # 1568 half plane
PW = W + 2                # 58 padded width
PLANE = PW * PW           # 3364 padded plane (58 rows x 58 cols)
PLANE_PAD = 3392          # plane stride, %16 == 0
BAND = 8                  # output rows per matmul
NBAND = H // BAND         # 7
NFREE = BAND * PW         # 464 psum free size per band
BANK = 512                # psum band stride (one 2KB bank)
NTOT = B * HW             # BN count (N*H*W over full batch)
EPS = 1e-5
OFFS = [(dh, dw) for dh in range(3) for dw in range(3)]
CHUNKS = ((0, 2), (2, 2), (4, 2), (6, 1))  # (first band, nbands) psum chunks

NTOT_ = float(NTOT)

# cvec column indices
CV_B1, CV_A1, CV_SF1N, CV_SF1SQN, CV_B2, CV_ISF1, CV_A2, \
    CV_SF2N, CV_SF2SQN, CV_G3SF2, CV_G3, CV_B3V, CV_A3, CV_EPS = range(14)
CV_NCOLS = 14

_CACHE = {}


def _build():
    nc = bacc.Bacc(num_devices=NCORES)
    x_d = nc.declare_dram_parameter("x", [BL, C, H, W], F32, isOutput=False)
    w1_d = nc.declare_dram_parameter("w1s", [128, 2, 18 * 128], F8, isOutput=False)
    w2_d = nc.declare_dram_parameter("w2s", [128, 2, 18 * 128], F8, isOutput=False)
    cv_d = nc.declare_dram_parameter("cvec", [128, 2, CV_NCOLS], F32, isOutput=False)
    out_d = nc.declare_dram_parameter("out", [BL, C, H, W], F16, isOutput=True)

    # DRAM-side views: channel c -> (g = c // 128, p = c % 128)
    def x_view(n):
        return x_d[n].rearrange("(g p) h w -> p g (h w)", p=128)

    def out_view(n):
        return out_d[n].rearrange("(g p) h w -> p g (h w)", p=128)

    with tile.TileContext(nc, num_cores=NCORES, pool_alloc_mode="queue") as tc:
        es_u1 = contextlib.ExitStack()
        es_u2 = contextlib.ExitStack()
        es_xs = contextlib.ExitStack()
        with tc.tile_pool(name="consts", bufs=1) as cpool, \
                tc.tile_pool(name="weights", bufs=1) as wpool, \
                tc.tile_pool(name="spool", bufs=1) as spool, \
                tc.tile_pool(name="sqscr", bufs=1) as sqpool, \
                tc.tile_pool(name="psum", bufs=1, space="PSUM") as psum_pool, \
                tc.tile_pool(name="dram", bufs=1, space="DRAM") as dram_pool, \
                es_u2:

            # ---- persistent small tiles ----
            cvec = cpool.tile([128, 2, CV_NCOLS], F32, tag="cvec")
            st1 = cpool.tile([128, 32], F32, tag="st1")
            st2 = cpool.tile([128, 112], F32, tag="st2")
            st3 = cpool.tile([128, 112], F32, tag="st3")
            g1 = cpool.tile([128, 4], F32, tag="g1")
            g2 = cpool.tile([128, 4], F32, tag="g2")
            g3t = cpool.tile([128, 4], F32, tag="g3t")
            negt1 = cpool.tile([128, 2], F32, tag="negt1")
            negth2 = cpool.tile([128, 2], F32, tag="negth2")
            kvec = cpool.tile([128, 2], F32, tag="kvec")
            dvec = cpool.tile([128, 2], F32, tag="dvec")
            tmp_a = cpool.tile([128, 2], F32, tag="tmp_a")
            tmp_b = cpool.tile([128, 2], F32, tag="tmp_b")
            tmp_c = cpool.tile([128, 2], F32, tag="tmp_c")

            w1t = wpool.tile([128, 2, 18 * 128], F8, tag="w1t")
            w2t = wpool.tile([128, 2, 18 * 128], F8, tag="w2t")

            s_tiles = [
                spool.tile([128, 2, PLANE_PAD], F8, tag="sa", name="sa"),
                spool.tile([128, 2, PLANE_PAD], F8, tag="sb", name="sb"),
            ]

            # shared square scratch (garbage out): sqs for ACT squares,
            # sqb (half-size) for GpSimd/DVE squares — separate tags so the
            # two engines' square chains don't serialize on WAW deps
            sqs = sqpool.tile([128, HW], F16, tag="sqs")
            sqb = sqpool.tile([128, HHW], BF16, tag="sqb")

            # persistent fp16 x (phase0 stats source + phase3 residual), u2
            x16_pool = es_u2.enter_context(tc.tile_pool(name="x16", bufs=BL))
            u2_pool = es_u2.enter_context(tc.tile_pool(name="u2", bufs=BL))
            u1_pool = es_u1.enter_context(tc.tile_pool(name="u1", bufs=BL))

            # =============== phase 0: fp16 x stream + stats ===============
            # st1 col j = (k*2 + g)*8 + n*2 + h  (k: 0=sum, 1=sumsq; h=half)
            # cast DMAs first on the gpsimd queue so their descriptor gen
            # isn't stuck behind the big s-tile memsets
            # stats-tile memsets first on the pool queue: the stats accums
            # wait on these (WAW), so they must precede casts and s-memsets
            nc.gpsimd.memset(st1[:], 0.0)
            nc.gpsimd.memset(st2[:], 0.0)
            nc.gpsimd.memset(st3[:], 0.0)
            x16s = []
            for n in range(BL):
                x16 = x16_pool.tile([128, 2, HW], F16, tag="x16", name=f"x16_{n}")
                x16s.append(x16)
                for g in range(2):
                    # cast f32 -> fp16 in flight (gpsimd DGE)
                    nc.gpsimd.dma_start(x16[:, g, :], x_view(n)[:, g, :])

            nc.sync.dma_start(cvec[:], cv_d[:])
            nc.sync.dma_start(w1t[:], w1_d[:])
            nc.sync.dma_start(w2t[:], w2_d[:])
            # only the pad borders of the sign planes need zeroing — the
            # interior is overwritten by every sign-prep (5 small memsets
            # per tile instead of an 11.5us full-plane pair)
            for s in s_tiles:
                nc.gpsimd.memset(s[:, :, 0:PW], 0.0)                    # row 0
                nc.gpsimd.memset(s[:, :, 57 * PW:58 * PW], 0.0)         # row 57
                sv = s[:, :, 0:PLANE].rearrange("p g (r w) -> p g r w", w=PW)
                nc.gpsimd.memset(sv[:, :, 1:57, 0:1], 0.0)              # col 0
                nc.gpsimd.memset(sv[:, :, 1:57, 57:58], 0.0)            # col 57
                nc.gpsimd.memset(s[:, :, PLANE:PLANE_PAD], 0.0)         # tail

            def s_plane(s, g):
                return s[:, g, 0:PLANE].rearrange("p (r w) -> p r w", w=PW)

            for n in range(BL):
                x16 = x16s[n]
                for g in range(2):
                    # sum(x) via in-place ts-accum (4x DVE mode)
                    nc.vector.tensor_scalar(
                        x16[:, g, :], x16[:, g, :], 1.0, 0.0,
                        op0=OP.mult, op1=OP.add,
                        accum_out=st1[:, (0 * 2 + g) * 8 + n * 2:(0 * 2 + g) * 8 + n * 2 + 1],
                    )
                    jq = (1 * 2 + g) * 8 + n * 2
                    if n == 1:
                        # pool square halves + DVE ts-accum: takes two of the
                        # eight squares off the serial ACT chain
                        for h in range(2):
                            xh = x16[:, g, h * HHW:(h + 1) * HHW]
                            nc.gpsimd.tensor_tensor(sqb[:], xh, xh, op=OP.mult)
                            nc.vector.tensor_scalar(
                                sqb[:], sqb[:], 1.0, 0.0, op0=OP.mult, op1=OP.add,
                                accum_out=st1[:, jq + h:jq + h + 1],
                            )
                    elif n < 3:
                        # ACT square, exact f32 accum
                        nc.scalar.activation(
                            sqs[:], x16[:, g, :], AF.Square, bias=0.0, scale=1.0,
                            accum_out=st1[:, jq:jq + 1],
                        )
                    else:
                        # last image: ACT halves keep the stats tail short
                        for h in range(2):
                            xh = x16[:, g, h * HHW:(h + 1) * HHW]
                            nc.scalar.activation(
                                sqs[:, 0:HHW], xh, AF.Square, bias=0.0,
                                scale=1.0, accum_out=st1[:, jq + h:jq + h + 1],
                            )

            # reduce st1 [128, (k g) 8] -> r1 [128, 4], AllGather -> g1
            r1 = cpool.tile([128, 4], F32, tag="r1")
            nc.vector.reduce_sum(
                r1[:].rearrange("p (a b) -> p a b", b=1),
                st1[:].rearrange("p (kg t) -> p kg t", t=2 * BL),
                axis=mybir.AxisListType.X,
            )
            ar1_i = dram_pool.tile([128, 4], F32, tag="ar1_i")
            ar1_o = dram_pool.tile([NCORES, 128, 4], F32, tag="ar1_o", addr_space="Shared")
            nc.sync.dma_start(ar1_i[:], r1[:])
            nc.gpsimd.collective_compute(
                "AllGather", OP.bypass, replica_groups=[list(range(NCORES))],
                ins=[ar1_i[:].opt()], outs=[ar1_o[:].opt()],
            )
            # gth read on the ACT hwdge queue so SP can run the conv1 x
            # re-stream during the collective
            gth1 = cpool.tile([128, 4, NCORES], F32, tag="gth1")
            nc.scalar.dma_start(gth1[:], ar1_o[:].rearrange("r p k -> p k r"))
            nc.vector.reduce_sum(
                g1[:].rearrange("p (a b) -> p a b", b=1), gth1[:],
                axis=mybir.AxisListType.X,
            )

            # conv1 f32 x re-stream (per half-image); transfers overlap AR1
            xs_pool = es_xs.enter_context(tc.tile_pool(name="xs", bufs=2))
            x1s = []
            for n in range(BL):
                halves = []
                for g in range(2):
                    xt = xs_pool.tile([128, HW], F32, tag="xt", name=f"x1_{n}_{g}")
                    nc.sync.dma_start(xt[:], x_view(n)[:, g, :])
                    halves.append(xt)
                x1s.append(halves)

            # ---- BN1 threshold: negt1 = B1*std1 - m1 ----
            g1v = g1[:].rearrange("p (k g) -> p k g", k=2)
            nc.vector.tensor_scalar_mul(tmp_a[:], g1v[:, 0], 1.0 / NTOT_)     # m1
            nc.vector.tensor_scalar_mul(tmp_b[:], g1v[:, 1], 1.0 / NTOT_)     # E[x^2]
            nc.vector.scalar_tensor_tensor(                                   # -m^2
                tmp_c[:], tmp_a[:], -1.0, tmp_a[:], op0=OP.mult, op1=OP.mult,
            )
            nc.vector.tensor_add(tmp_c[:], tmp_c[:], tmp_b[:])                # v1
            nc.scalar.activation(tmp_b[:], tmp_c[:], AF.Sqrt, bias=cvec[:, 0, CV_EPS:CV_EPS + 1], scale=1.0)  # std1
            nc.vector.tensor_mul(tmp_c[:], tmp_b[:], cvec[:, :, CV_B1])       # B1*std1
            nc.vector.tensor_sub(negt1[:], tmp_c[:], tmp_a[:])                # B1*std1 - m1

            # =============== conv pass helper ===============
            def conv_pass(widx, wt, stats, prep, u_pool, a_col):
                """One binary conv over all images.

                prep(n, s, part) emits the sign-write of image n into s:
                part 0 = image rows 0..31 (enough for bands 0-2), part 1 =
                rows 32..55 (handled by prep itself; called with both parts
                back to back except for image 0 where part 0 is early).

                stats col j = (k*2+cc)*28 + n*7 + q  (k: 0=sum u, 1=sum u^2;
                q = chunk/half index, unused cols stay zero).
                Returns list of u tiles [128, 2, H, W] fp16 (y-units).
                """
                u_tiles = []
                prep(0, s_tiles[0], 0)
                prep(0, s_tiles[0], 1)
                for n in range(BL):
                    s = s_tiles[n % 2]
                    ut = u_pool.tile([128, 2, H, W], F16, tag=f"u{widx}", name=f"u{widx}_{n}")
                    u_tiles.append(ut)
                    last = n == BL - 1
                    for cc in range(2):
                        for ci, (b0, nb) in enumerate(CHUNKS):
                            pt = psum_pool.tile(
                                [128, nb, BANK], F32,
                                tag="pt2" if nb == 2 else "pt1",
                                bufs=3 if nb == 2 else 2,
                                name=f"pt{widx}_{n}_{cc}_{ci}",
                            )
                            for k in range(nb):
                                b = b0 + k
                                po = pt[:, k, 0:NFREE]
                                for o, (dh, dw) in enumerate(OFFS):
                                    start = (b * BAND + dh) * PW + dw
                                    nc.tensor.matmul(
                                        po,
                                        wt[:, :, (o * 2 + cc) * 128:(o * 2 + cc + 1) * 128],
                                        s[:, :, start:start + NFREE],
                                        start=(o == 0), stop=(o == 8),
                                        perf_mode=DR,
                                    )
                            # copy psum chunk -> u rows (strided 4D read);
                            # cc0 on ACT, cc1 on DVE to balance engine load.
                            # The copy is the psum bank's only reader, so
                            # banks recycle without waiting on the prelu.
                            pv = pt[:, :, 0:NFREE].rearrange(
                                "p k (r w) -> p k r w", w=PW)[:, :, :, 0:W]
                            us = ut[:, cc, b0 * BAND:(b0 + nb) * BAND, :]
                            ud = us.rearrange("p (k r) w -> p k r w", k=nb)
                            if cc == 0:
                                nc.scalar.activation(
                                    ud, pv, AF.Identity, bias=0.0, scale=1.0,
                                )
                            else:
                                nc.vector.tensor_scalar(
                                    ud, pv, 1.0, None, op0=OP.mult,
                                )
                            # u = max(y, a*y) in place per chunk, accum Σu
                            j0 = (0 * 2 + cc) * 28 + n * NBAND + ci
                            nc.vector.scalar_tensor_tensor(
                                us, us, cvec[:, cc, a_col:a_col + 1], us,
                                op0=OP.mult, op1=OP.max,
                                accum_out=stats[:, j0:j0 + 1],
                            )
                        # next image's sign-prep part between the cc phases:
                        # during cc1 the ACT queue is otherwise free
                        if n + 1 < BL:
                            prep(n + 1, s_tiles[(n + 1) % 2], cc)
                    # sum(u^2) per (cc, n) at image end: ACT when it has
                    # slack, GpSimd+DVE otherwise; last image on ACT halves
                    # so the stats tail after the final band stays short
                    for cc in range(2):
                        j1 = (1 * 2 + cc) * 28 + n * NBAND
                        uf = ut[:, cc, :, :].rearrange("p h w -> p (h w)")
                        act_sq = (cc == 0 and n != 1) or last
                        if act_sq and not last:
                            nc.scalar.activation(
                                sqs[:], uf, AF.Square, bias=0.0, scale=1.0,
                                accum_out=stats[:, j1:j1 + 1],
                            )
                        elif act_sq:
                            # last image: quarter squares so only the final
                            # quarter trails the last band
                            q4 = HW // 4
                            for h in range(4):
                                nc.scalar.activation(
                                    sqs[:, 0:q4], uf[:, h * q4:(h + 1) * q4],
                                    AF.Square, bias=0.0, scale=1.0,
                                    accum_out=stats[:, j1 + h:j1 + h + 1],
                                )
                        else:
                            for h in range(2):
                                uh = uf[:, h * HHW:(h + 1) * HHW]
                                nc.gpsimd.tensor_tensor(sqb[:], uh, uh, op=OP.mult)
                                nc.vector.tensor_scalar(
                                    sqb[:], sqb[:], 1.0, 0.0,
                                    op0=OP.mult, op1=OP.add,
                                    accum_out=stats[:, j1 + h:j1 + h + 1],
                                )
                return u_tiles

            # =============== conv1 ===============
            def prep1(n, s, part):
                r0, r1_ = (0, 32) if part == 0 else (32, 56)
                for g in range(2):
                    nc.scalar.activation(
                        s_plane(s, g)[:, 1 + r0:1 + r1_, 1:57],
                        x1s[n][g].rearrange("p (h w) -> p h w", w=W)[:, r0:r1_, :],
                        AF.Sign, bias=negt1[:, g:g + 1], scale=1.0,
                    )

            u1 = conv_pass(0, w1t, st2, prep1, u1_pool, CV_A1)
            es_xs.close()  # f32 x stream fully consumed

            # reduce st2 -> r2, AllGather -> g2
            r2 = cpool.tile([128, 4], F32, tag="r2")
            nc.vector.reduce_sum(
                r2[:].rearrange("p (a b) -> p a b", b=1),
                st2[:].rearrange("p (kc t) -> p kc t", t=28),
                axis=mybir.AxisListType.X,
            )
            ar2_i = dram_pool.tile([128, 4], F32, tag="ar2_i")
            ar2_o = dram_pool.tile([NCORES, 128, 4], F32, tag="ar2_o", addr_space="Shared")
            nc.sync.dma_start(ar2_i[:], r2[:])
            nc.gpsimd.collective_compute(
                "AllGather", OP.bypass, replica_groups=[list(range(NCORES))],
                ins=[ar2_i[:].opt()], outs=[ar2_o[:].opt()],
            )
            gth2 = cpool.tile([128, 4, NCORES], F32, tag="gth2")
            nc.scalar.dma_start(gth2[:], ar2_o[:].rearrange("r p k -> p k r"))
            nc.vector.reduce_sum(
                g2[:].rearrange("p (a b) -> p a b", b=1), gth2[:],
                axis=mybir.AxisListType.X,
            )

            # ---- BN2 threshold in u1 units: negth2 = (B2*std2 - m2)/sf1 ----
            g2v = g2[:].rearrange("p (k c) -> p k c", k=2)
            nc.vector.tensor_mul(tmp_a[:], g2v[:, 0], cvec[:, :, CV_SF1N])    # m2
            nc.vector.tensor_mul(tmp_b[:], g2v[:, 1], cvec[:, :, CV_SF1SQN])  # E[p1^2]
            nc.vector.scalar_tensor_tensor(
                tmp_c[:], tmp_a[:], -1.0, tmp_a[:], op0=OP.mult, op1=OP.mult,
            )
            nc.vector.tensor_add(tmp_c[:], tmp_c[:], tmp_b[:])                # v2
            nc.scalar.activation(tmp_b[:], tmp_c[:], AF.Sqrt, bias=cvec[:, 0, CV_EPS:CV_EPS + 1], scale=1.0)  # std2
            nc.vector.tensor_mul(tmp_c[:], tmp_b[:], cvec[:, :, CV_B2])       # B2*std2
            nc.vector.tensor_sub(tmp_a[:], tmp_a[:], tmp_c[:])                # t2 = m2 - B2*std2
            nc.vector.tensor_mul(tmp_a[:], tmp_a[:], cvec[:, :, CV_ISF1])     # theta (u units)
            nc.vector.tensor_scalar_mul(negth2[:], tmp_a[:], -1.0)

            # =============== conv2 ===============
            def prep2(n, s, part):
                r0, r1_ = (0, 32) if part == 0 else (32, 56)
                for g in range(2):
                    nc.scalar.activation(
                        s_plane(s, g)[:, 1 + r0:1 + r1_, 1:57],
                        u1[n][:, g, r0:r1_, :],
                        AF.Sign, bias=negth2[:, g:g + 1], scale=1.0,
                    )

            u2 = conv_pass(1, w2t, st3, prep2, u2_pool, CV_A2)

            # u1 fully consumed by prep2; release its pool
            es_u1.close()

            # reduce st3 -> r3, AllGather -> g3t
            r3 = cpool.tile([128, 4], F32, tag="r3")
            nc.vector.reduce_sum(
                r3[:].rearrange("p (a b) -> p a b", b=1),
                st3[:].rearrange("p (kc t) -> p kc t", t=28),
                axis=mybir.AxisListType.X,
            )
            ar3_i = dram_pool.tile([128, 4], F32, tag="ar3_i")
            ar3_o = dram_pool.tile([NCORES, 128, 4], F32, tag="ar3_o", addr_space="Shared")
            nc.sync.dma_start(ar3_i[:], r3[:])
            nc.gpsimd.collective_compute(
                "AllGather", OP.bypass, replica_groups=[list(range(NCORES))],
                ins=[ar3_i[:].opt()], outs=[ar3_o[:].opt()],
            )
            gth3 = cpool.tile([128, 4, NCORES], F32, tag="gth3")
            nc.scalar.dma_start(gth3[:], ar3_o[:].rearrange("r p k -> p k r"))
            nc.vector.reduce_sum(
                g3t[:].rearrange("p (a b) -> p a b", b=1), gth3[:],
                axis=mybir.AxisListType.X,
            )

            # ---- BN3 affine: K = g3*sf2*rstd3, D = b3 - m3*g3*rstd3 ----
            g3v = g3t[:].rearrange("p (k c) -> p k c", k=2)
            nc.vector.tensor_mul(tmp_a[:], g3v[:, 0], cvec[:, :, CV_SF2N])    # m3
            nc.vector.tensor_mul(tmp_b[:], g3v[:, 1], cvec[:, :, CV_SF2SQN])  # E[p2^2]
            nc.vector.scalar_tensor_tensor(
                tmp_c[:], tmp_a[:], -1.0, tmp_a[:], op0=OP.mult, op1=OP.mult,
            )
            nc.vector.tensor_add(tmp_c[:], tmp_c[:], tmp_b[:])                # v3
            nc.scalar.activation(tmp_b[:], tmp_c[:], AF.Sqrt, bias=cvec[:, 0, CV_EPS:CV_EPS + 1], scale=1.0)  # std3
            nc.vector.reciprocal(tmp_c[:], tmp_b[:])                          # rstd3
            nc.vector.tensor_mul(kvec[:], tmp_c[:], cvec[:, :, CV_G3SF2])     # K
            nc.vector.tensor_mul(tmp_a[:], tmp_a[:], cvec[:, :, CV_G3])       # m3*g3
            nc.vector.tensor_mul(tmp_a[:], tmp_a[:], tmp_c[:])                # m3*g3*rstd3
            nc.vector.tensor_sub(dvec[:], cvec[:, :, CV_B3V], tmp_a[:])       # D

            # ====== phase 3: out = prelu(K*u2 + D + x, a3), fp16 out ======
            with tc.tile_pool(name="ph3", bufs=2) as p3pool:
                for n in range(BL):
                    th = p3pool.tile([128, 2, HW], F16, tag="th", name=f"th_{n}")
                    ot = p3pool.tile([128, 2, HW], F16, tag="ot", name=f"ot_{n}")
                    for g in range(2):
                        nc.scalar.activation(
                            th[:, g, :],
                            u2[n][:, g, :, :].rearrange("p h w -> p (h w)"),
                            AF.Identity,
                            bias=dvec[:, g:g + 1], scale=kvec[:, g:g + 1],
                        )
                    for g in range(2):
                        # w = th + x (2x tt), aw = a3*w (4x ts),
                        # out = max(w, aw) (2x tt) — per-g DVE pipeline
                        nc.vector.tensor_tensor(
                            th[:, g, :], th[:, g, :], x16s[n][:, g, :], op=OP.add,
                        )
                        nc.vector.tensor_scalar(
                            ot[:, g, :], th[:, g, :],
                            cvec[:, g, CV_A3:CV_A3 + 1], None, op0=OP.mult,
                        )
                        nc.vector.tensor_tensor(
                            ot[:, g, :], ot[:, g, :], th[:, g, :], op=OP.max,
                        )
                    nc.sync.dma_start(out_view(n), ot[:])

    nc.compile()
    return nc


def _host_prep(inputs):
    x = np.ascontiguousarray(np.asarray(inputs["x"], dtype=np.float32))
    w1 = np.asarray(inputs["w1"], dtype=np.float32)
    w2 = np.asarray(inputs["w2"], dtype=np.float32)

    def wprep(w):
        ws = np.sign(w).astype(np.float32)  # [co, ci, kh, kw]
        sf = np.abs(w).mean(axis=(1, 2, 3)).astype(np.float32)  # [256]
        arr = np.empty((128, 2, 18, 128), dtype=np.float32)
        for o, (dh, dw) in enumerate(OFFS):
            for cc in range(2):
                t = ws[cc * 128:(cc + 1) * 128, :, dh, dw]  # [m, ci]
                # arr[p, g, blk, m] = t[m, g*128 + p]
                arr[:, :, o * 2 + cc, :] = t.T.reshape(2, 128, 128).transpose(1, 0, 2)
        return arr.reshape(128, 2, 18 * 128).astype(F8NP), sf

    w1s, sf1 = wprep(w1)
    w2s, sf2 = wprep(w2)

    def vec(v):
        return np.asarray(v, dtype=np.float32).reshape(2, 128).T  # [p, g]

    g1v, b1v = inputs["g1"], inputs["b1"]
    g2v, b2v = inputs["g2"], inputs["b2"]
    g3v, b3v = inputs["g3"], inputs["b3"]
    a1, a2, a3 = inputs["a1"], inputs["a2"], inputs["a3"]

    cvec = np.zeros((128, 2, CV_NCOLS), dtype=np.float32)
    cvec[:, :, CV_B1] = vec(np.asarray(b1v) / np.asarray(g1v))
    cvec[:, :, CV_A1] = vec(np.asarray(a1))
    cvec[:, :, CV_SF1N] = vec(sf1 / NTOT_)
    cvec[:, :, CV_SF1SQN] = vec(sf1 * sf1 / NTOT_)
    cvec[:, :, CV_B2] = vec(np.asarray(b2v) / np.asarray(g2v))
    cvec[:, :, CV_ISF1] = vec(1.0 / sf1)
    cvec[:, :, CV_A2] = vec(np.asarray(a2))
    cvec[:, :, CV_SF2N] = vec(sf2 / NTOT_)
    cvec[:, :, CV_SF2SQN] = vec(sf2 * sf2 / NTOT_)
    cvec[:, :, CV_G3SF2] = vec(np.asarray(g3v) * sf2)
    cvec[:, :, CV_G3] = vec(np.asarray(g3v))
    cvec[:, :, CV_B3V] = vec(np.asarray(b3v))
    cvec[:, :, CV_A3] = vec(np.asarray(a3))
    cvec[:, :, CV_EPS] = EPS

    return x, w1s, w2s, cvec


def run(inputs, trace=False):
    x, w1s, w2s, cvec = _host_prep(inputs)
    if "nc" not in _CACHE:
        _CACHE["nc"] = _build()
    nc = _CACHE["nc"]
    in_maps = [
        {"x": x[i * BL:(i + 1) * BL], "w1s": w1s, "w2s": w2s, "cvec": cvec}
        for i in range(NCORES)
    ]
    res = run_bass_kernel_spmd(nc, in_maps, list(range(NCORES)), trace=trace)
    out = np.concatenate([res.results[i]["out"] for i in range(NCORES)], axis=0)
    return out.astype(np.float32), res


def kernel(**inputs):
    out, _ = run(inputs, trace=False)
    return out


if __name__ == "__main__":
    # build-only check
    _build()
    print("BUILD OK")


# revision 30
# speedup vs baseline: 1.1011x; 1.0341x over previous
"""XNOR-Net BasicBlock (BN-sign-conv x2 + residual, training-mode BN) on 8 TRN2 cores.

Strategy (data-parallel on batch, 4 images/core):
  phase0: x streamed fp16 (gpsimd cast DMA, kept for the phase-3 residual);
          per-channel sum via DVE ts-accum, sumsq via ACT Square / DVE
          tensor_tensor_reduce -> AllGather (BN1 stats)
  conv1 : x re-streamed f32 (overlaps AR1) for exact s1 = sign(x - t1) fp8;
          3x3 conv as 9 DoubleRow fp8 matmuls per 8-row band into 4-band
          PSUM tiles; epilogue per 4/3-band chunk: DVE copy psum->u (fp16),
          in-place DVE stt u = max(y, a*y) with accum -> sum(u);
          sum(u^2): cc0 via ACT Square+accum, cc1 via GpSimd square +
          DVE ts-accum  -> AllGather (BN2 stats)
  conv2 : s2 = sign(u1 - theta2), same -> AllGather (BN3 stats)
  phase3: out = prelu(K*u2 + D + x, a3) in fp16 (host casts to f32)
"""

import sys

sys.path.insert(0, "/opt/trn_rl_repo")

import contextlib

import numpy as np

import concourse.bacc as bacc
import concourse.mybir as mybir
import concourse.tile as tile
from concourse.bass_utils import run_bass_kernel_spmd

F32 = mybir.dt.float32
F16 = mybir.dt.float16
BF16 = mybir.dt.bfloat16
F8 = mybir.dt.float8e4
F8NP = mybir.dt.np(F8)

AF = mybir.ActivationFunctionType
OP = mybir.AluOpType
DR = mybir.MatmulPerfMode.DoubleRow

NCORES = 8
B, C, H, W = 32, 256, 56, 56
BL = B // NCORES          # images per core
HW = H * W                # 3136
HHW = HW // 2             # 1568 half plane
PW = W + 2                # 58 padded width
PLANE = PW * PW           # 3364 padded plane (58 rows x 58 cols)
PLANE_PAD = 3392          # plane stride, %16 == 0
BAND = 8                  # output rows per matmul
NBAND = H // BAND         # 7
NFREE = BAND * PW         # 464 psum free size per band
BANK = 512                # psum band stride (one 2KB bank)
NTOT = B * HW             # BN count (N*H*W over full batch)
EPS = 1e-5
OFFS = [(dh, dw) for dh in range(3) for dw in range(3)]
CHUNKS = ((0, 2), (2, 2), (4, 2), (6, 1))  # (first band, nbands) psum chunks

NTOT_ = float(NTOT)

# cvec column indices
CV_B1, CV_A1, CV_SF1N, CV_SF1SQN, CV_B2, CV_ISF1, CV_A2, \
    CV_SF2N, CV_SF2SQN, CV_G3SF2, CV_G3, CV_B3V, CV_A3, CV_EPS = range(14)
CV_NCOLS = 14

_CACHE = {}


def _build():
    nc = bacc.Bacc(num_devices=NCORES)
    x_d = nc.declare_dram_parameter("x", [BL, C, H, W], F32, isOutput=False)
    w1_d = nc.declare_dram_parameter("w1s", [128, 2, 18 * 128], F8, isOutput=False)
    w2_d = nc.declare_dram_parameter("w2s", [128, 2, 18 * 128], F8, isOutput=False)
    cv_d = nc.declare_dram_parameter("cvec", [128, 2, CV_NCOLS], F32, isOutput=False)
    out_d = nc.declare_dram_parameter("out", [BL, C, H, W], F16, isOutput=True)

    # DRAM-side views: channel c -> (g = c // 128, p = c % 128)
    def x_view(n):
        return x_d[n].rearrange("(g p) h w -> p g (h w)", p=128)

    def out_view(n):
        return out_d[n].rearrange("(g p) h w -> p g (h w)", p=128)

    with tile.TileContext(nc, num_cores=NCORES, pool_alloc_mode="queue") as tc:
        es_u1 = contextlib.ExitStack()
        es_u2 = contextlib.ExitStack()
        es_xs = contextlib.ExitStack()
        with tc.tile_pool(name="consts", bufs=1) as cpool, \
                tc.tile_pool(name="weights", bufs=1) as wpool, \
                tc.tile_pool(name="spool", bufs=1) as spool, \
                tc.tile_pool(name="sqscr", bufs=1) as sqpool, \
                tc.tile_pool(name="psum", bufs=1, space="PSUM") as psum_pool, \
                tc.tile_pool(name="dram", bufs=1, space="DRAM") as dram_pool, \
                es_u2:

            # ---- persistent small tiles ----
            cvec = cpool.tile([128, 2, CV_NCOLS], F32, tag="cvec")
            st1 = cpool.tile([128, 32], F32, tag="st1")
            st2 = cpool.tile([128, 112], F32, tag="st2")
            st3 = cpool.tile([128, 112], F32, tag="st3")
            g1 = cpool.tile([128, 4], F32, tag="g1")
            g2 = cpool.tile([128, 4], F32, tag="g2")
            g3t = cpool.tile([128, 4], F32, tag="g3t")
            negt1 = cpool.tile([128, 2], F32, tag="negt1")
            negth2 = cpool.tile([128, 2], F32, tag="negth2")
            kvec = cpool.tile([128, 2], F32, tag="kvec")
            dvec = cpool.tile([128, 2], F32, tag="dvec")
            tmp_a = cpool.tile([128, 2], F32, tag="tmp_a")
            tmp_b = cpool.tile([128, 2], F32, tag="tmp_b")
            tmp_c = cpool.tile([128, 2], F32, tag="tmp_c")

            w1t = wpool.tile([128, 2, 18 * 128], F8, tag="w1t")
            w2t = wpool.tile([128, 2, 18 * 128], F8, tag="w2t")

            s_tiles = [
                spool.tile([128, 2, PLANE_PAD], F8, tag="sa", name="sa"),
                spool.tile([128, 2, PLANE_PAD], F8, tag="sb", name="sb"),
            ]

            # shared square scratch (garbage out): sqs for ACT squares,
            # sqb (half-size) for GpSimd/DVE squares — separate tags so the
            # two engines' square chains don't serialize on WAW deps
            sqs = sqpool.tile([128, HW], F16, tag="sqs")
            sqb = sqpool.tile([128, HHW], BF16, tag="sqb")

            # persistent fp16 x (phase0 stats source + phase3 residual), u2
            x16_pool = es_u2.enter_context(tc.tile_pool(name="x16", bufs=BL))
            u2_pool = es_u2.enter_context(tc.tile_pool(name="u2", bufs=BL))
            u1_pool = es_u1.enter_context(tc.tile_pool(name="u1", bufs=BL))

            # =============== phase 0: fp16 x stream + stats ===============
            # st1 col j = (k*2 + g)*8 + n*2 + h  (k: 0=sum, 1=sumsq; h=half)
            # cast DMAs first on the gpsimd queue so their descriptor gen
            # isn't stuck behind the big s-tile memsets
            # stats-tile memsets first on the pool queue: the stats accums
            # wait on these (WAW), so they must precede casts and s-memsets
            nc.gpsimd.memset(st1[:], 0.0)
            nc.gpsimd.memset(st2[:], 0.0)
            nc.gpsimd.memset(st3[:], 0.0)
            x16s = []
            for n in range(BL):
                x16 = x16_pool.tile([128, 2, HW], F16, tag="x16", name=f"x16_{n}")
                x16s.append(x16)
                for g in range(2):
                    # cast f32 -> fp16 in flight (gpsimd DGE)
                    nc.gpsimd.dma_start(x16[:, g, :], x_view(n)[:, g, :])

            nc.sync.dma_start(cvec[:], cv_d[:])
            nc.sync.dma_start(w1t[:], w1_d[:])
            nc.sync.dma_start(w2t[:], w2_d[:])
            # only the pad borders of the sign planes need zeroing — the
            # interior is overwritten by every sign-prep (5 small memsets
            # per tile instead of an 11.5us full-plane pair)
            for s in s_tiles:
                nc.gpsimd.memset(s[:, :, 0:PW], 0.0)                    # row 0
                nc.gpsimd.memset(s[:, :, 57 * PW:58 * PW], 0.0)         # row 57
                sv = s[:, :, 0:PLANE].rearrange("p g (r w) -> p g r w", w=PW)
                nc.gpsimd.memset(sv[:, :, 1:57, 0:1], 0.0)              # col 0
                nc.gpsimd.memset(sv[:, :, 1:57, 57:58], 0.0)            # col 57
                nc.gpsimd.memset(s[:, :, PLANE:PLANE_PAD], 0.0)         # tail

            def s_plane(s, g):
                return s[:, g, 0:PLANE].rearrange("p (r w) -> p r w", w=PW)

            for n in range(BL):
                x16 = x16s[n]
                for g in range(2):
                    # sum(x) via in-place ts-accum (4x DVE mode)
                    nc.vector.tensor_scalar(
                        x16[:, g, :], x16[:, g, :], 1.0, 0.0,
                        op0=OP.mult, op1=OP.add,
                        accum_out=st1[:, (0 * 2 + g) * 8 + n * 2:(0 * 2 + g) * 8 + n * 2 + 1],
                    )
                    jq = (1 * 2 + g) * 8 + n * 2
                    if n < 3:
                        # ACT square, exact f32 accum
                        nc.scalar.activation(
                            sqs[:], x16[:, g, :], AF.Square, bias=0.0, scale=1.0,
                            accum_out=st1[:, jq:jq + 1],
                        )
                    else:
                        # last image: ACT halves keep the stats tail short
                        for h in range(2):
                            xh = x16[:, g, h * HHW:(h + 1) * HHW]
                            nc.scalar.activation(
                                sqs[:, 0:HHW], xh, AF.Square, bias=0.0,
                                scale=1.0, accum_out=st1[:, jq + h:jq + h + 1],
                            )

            # reduce st1 [128, (k g) 8] -> r1 [128, 4], AllGather -> g1
            r1 = cpool.tile([128, 4], F32, tag="r1")
            nc.vector.reduce_sum(
                r1[:].rearrange("p (a b) -> p a b", b=1),
                st1[:].rearrange("p (kg t) -> p kg t", t=2 * BL),
                axis=mybir.AxisListType.X,
            )
            ar1_i = dram_pool.tile([128, 4], F32, tag="ar1_i")
            ar1_o = dram_pool.tile([NCORES, 128, 4], F32, tag="ar1_o", addr_space="Shared")
            nc.sync.dma_start(ar1_i[:], r1[:])
            nc.gpsimd.collective_compute(
                "AllGather", OP.bypass, replica_groups=[list(range(NCORES))],
                ins=[ar1_i[:].opt()], outs=[ar1_o[:].opt()],
            )
            # gth read on the ACT hwdge queue so SP can run the conv1 x
            # re-stream during the collective
            gth1 = cpool.tile([128, 4, NCORES], F32, tag="gth1")
            nc.scalar.dma_start(gth1[:], ar1_o[:].rearrange("r p k -> p k r"))
            nc.vector.reduce_sum(
                g1[:].rearrange("p (a b) -> p a b", b=1), gth1[:],
                axis=mybir.AxisListType.X,
            )

            # conv1 f32 x re-stream (per half-image); transfers overlap AR1
            xs_pool = es_xs.enter_context(tc.tile_pool(name="xs", bufs=2))
            x1s = []
            for n in range(BL):
                halves = []
                for g in range(2):
                    xt = xs_pool.tile([128, HW], F32, tag="xt", name=f"x1_{n}_{g}")
                    # gpsimd queue: descriptor generation runs after the
                    # phase-0 cast DMAs, so the casts keep DMA priority
                    nc.gpsimd.dma_start(xt[:], x_view(n)[:, g, :])
                    halves.append(xt)
                x1s.append(halves)

            # ---- BN1 threshold: negt1 = B1*std1 - m1 ----
            g1v = g1[:].rearrange("p (k g) -> p k g", k=2)
            nc.vector.tensor_scalar_mul(tmp_a[:], g1v[:, 0], 1.0 / NTOT_)     # m1
            nc.vector.tensor_scalar_mul(tmp_b[:], g1v[:, 1], 1.0 / NTOT_)     # E[x^2]
            nc.vector.scalar_tensor_tensor(                                   # -m^2
                tmp_c[:], tmp_a[:], -1.0, tmp_a[:], op0=OP.mult, op1=OP.mult,
            )
            nc.vector.tensor_add(tmp_c[:], tmp_c[:], tmp_b[:])                # v1
            nc.scalar.activation(tmp_b[:], tmp_c[:], AF.Sqrt, bias=cvec[:, 0, CV_EPS:CV_EPS + 1], scale=1.0)  # std1
            nc.vector.tensor_mul(tmp_c[:], tmp_b[:], cvec[:, :, CV_B1])       # B1*std1
            nc.vector.tensor_sub(negt1[:], tmp_c[:], tmp_a[:])                # B1*std1 - m1

            # =============== conv pass helper ===============
            def conv_pass(widx, wt, stats, prep, u_pool, a_col):
                """One binary conv over all images.

                prep(n, s, part) emits the sign-write of image n into s:
                part 0 = image rows 0..31 (enough for bands 0-2), part 1 =
                rows 32..55 (handled by prep itself; called with both parts
                back to back except for image 0 where part 0 is early).

                stats col j = (k*2+cc)*28 + n*7 + q  (k: 0=sum u, 1=sum u^2;
                q = chunk/half index, unused cols stay zero).
                Returns list of u tiles [128, 2, H, W] fp16 (y-units).
                """
                u_tiles = []
                prep(0, s_tiles[0], 0)
                prep(0, s_tiles[0], 1)
                for n in range(BL):
                    s = s_tiles[n % 2]
                    ut = u_pool.tile([128, 2, H, W], F16, tag=f"u{widx}", name=f"u{widx}_{n}")
                    u_tiles.append(ut)
                    last = n == BL - 1
                    for cc in range(2):
                        for ci, (b0, nb) in enumerate(CHUNKS):
                            pt = psum_pool.tile(
                                [128, nb, BANK], F32,
                                tag="pt2" if nb == 2 else "pt1",
                                bufs=3 if nb == 2 else 2,
                                name=f"pt{widx}_{n}_{cc}_{ci}",
                            )
                            for k in range(nb):
                                b = b0 + k
                                po = pt[:, k, 0:NFREE]
                                for o, (dh, dw) in enumerate(OFFS):
                                    start = (b * BAND + dh) * PW + dw
                                    nc.tensor.matmul(
                                        po,
                                        wt[:, :, (o * 2 + cc) * 128:(o * 2 + cc + 1) * 128],
                                        s[:, :, start:start + NFREE],
                                        start=(o == 0), stop=(o == 8),
                                        perf_mode=DR,
                                    )
                            # copy psum chunk -> u rows (strided 4D read);
                            # cc0 on ACT, cc1 on DVE to balance engine load.
                            # The copy is the psum bank's only reader, so
                            # banks recycle without waiting on the prelu.
                            pv = pt[:, :, 0:NFREE].rearrange(
                                "p k (r w) -> p k r w", w=PW)[:, :, :, 0:W]
                            us = ut[:, cc, b0 * BAND:(b0 + nb) * BAND, :]
                            ud = us.rearrange("p (k r) w -> p k r w", k=nb)
                            if cc == 0:
                                nc.scalar.activation(
                                    ud, pv, AF.Identity, bias=0.0, scale=1.0,
                                )
                            else:
                                nc.vector.tensor_scalar(
                                    ud, pv, 1.0, None, op0=OP.mult,
                                )
                            # u = max(y, a*y) in place per chunk, accum Σu
                            j0 = (0 * 2 + cc) * 28 + n * NBAND + ci
                            nc.vector.scalar_tensor_tensor(
                                us, us, cvec[:, cc, a_col:a_col + 1], us,
                                op0=OP.mult, op1=OP.max,
                                accum_out=stats[:, j0:j0 + 1],
                            )
                        # next image's sign-prep part between the cc phases:
                        # during cc1 the ACT queue is otherwise free
                        if n + 1 < BL:
                            prep(n + 1, s_tiles[(n + 1) % 2], cc)
                    # sum(u^2) per (cc, n) at image end: ACT when it has
                    # slack, GpSimd+DVE otherwise; last image on ACT halves
                    # so the stats tail after the final band stays short
                    for cc in range(2):
                        j1 = (1 * 2 + cc) * 28 + n * NBAND
                        uf = ut[:, cc, :, :].rearrange("p h w -> p (h w)")
                        act_sq = (cc == 0 and n != 1) or last
                        if act_sq and not last:
                            nc.scalar.activation(
                                sqs[:], uf, AF.Square, bias=0.0, scale=1.0,
                                accum_out=stats[:, j1:j1 + 1],
                            )
                        elif act_sq:
                            # last image: quarter squares so only the final
                            # quarter trails the last band
                            q4 = HW // 4
                            for h in range(4):
                                nc.scalar.activation(
                                    sqs[:, 0:q4], uf[:, h * q4:(h + 1) * q4],
                                    AF.Square, bias=0.0, scale=1.0,
                                    accum_out=stats[:, j1 + h:j1 + h + 1],
                                )
                        else:
                            for h in range(2):
                                uh = uf[:, h * HHW:(h + 1) * HHW]
                                nc.gpsimd.tensor_tensor(sqb[:], uh, uh, op=OP.mult)
                                nc.vector.tensor_scalar(
                                    sqb[:], sqb[:], 1.0, 0.0,
                                    op0=OP.mult, op1=OP.add,
                                    accum_out=stats[:, j1 + h:j1 + h + 1],
                                )
                return u_tiles

            # =============== conv1 ===============
            def prep1(n, s, part):
                r0, r1_ = (0, 32) if part == 0 else (32, 56)
                for g in range(2):
                    nc.scalar.activation(
                        s_plane(s, g)[:, 1 + r0:1 + r1_, 1:57],
                        x1s[n][g].rearrange("p (h w) -> p h w", w=W)[:, r0:r1_, :],
                        AF.Sign, bias=negt1[:, g:g + 1], scale=1.0,
                    )

            u1 = conv_pass(0, w1t, st2, prep1, u1_pool, CV_A1)
            es_xs.close()  # f32 x stream fully consumed

            # reduce st2 -> r2, AllGather -> g2
            r2 = cpool.tile([128, 4], F32, tag="r2")
            nc.vector.reduce_sum(
                r2[:].rearrange("p (a b) -> p a b", b=1),
                st2[:].rearrange("p (kc t) -> p kc t", t=28),
                axis=mybir.AxisListType.X,
            )
            ar2_i = dram_pool.tile([128, 4], F32, tag="ar2_i")
            ar2_o = dram_pool.tile([NCORES, 128, 4], F32, tag="ar2_o", addr_space="Shared")
            nc.sync.dma_start(ar2_i[:], r2[:])
            nc.gpsimd.collective_compute(
                "AllGather", OP.bypass, replica_groups=[list(range(NCORES))],
                ins=[ar2_i[:].opt()], outs=[ar2_o[:].opt()],
            )
            gth2 = cpool.tile([128, 4, NCORES], F32, tag="gth2")
            nc.scalar.dma_start(gth2[:], ar2_o[:].rearrange("r p k -> p k r"))
            nc.vector.reduce_sum(
                g2[:].rearrange("p (a b) -> p a b", b=1), gth2[:],
                axis=mybir.AxisListType.X,
            )

            # ---- BN2 threshold in u1 units: negth2 = (B2*std2 - m2)/sf1 ----
            g2v = g2[:].rearrange("p (k c) -> p k c", k=2)
            nc.vector.tensor_mul(tmp_a[:], g2v[:, 0], cvec[:, :, CV_SF1N])    # m2
            nc.vector.tensor_mul(tmp_b[:], g2v[:, 1], cvec[:, :, CV_SF1SQN])  # E[p1^2]
            nc.vector.scalar_tensor_tensor(
                tmp_c[:], tmp_a[:], -1.0, tmp_a[:], op0=OP.mult, op1=OP.mult,
            )
            nc.vector.tensor_add(tmp_c[:], tmp_c[:], tmp_b[:])                # v2
            nc.scalar.activation(tmp_b[:], tmp_c[:], AF.Sqrt, bias=cvec[:, 0, CV_EPS:CV_EPS + 1], scale=1.0)  # std2
            nc.vector.tensor_mul(tmp_c[:], tmp_b[:], cvec[:, :, CV_B2])       # B2*std2
            nc.vector.tensor_sub(tmp_a[:], tmp_a[:], tmp_c[:])                # t2 = m2 - B2*std2
            nc.vector.tensor_mul(tmp_a[:], tmp_a[:], cvec[:, :, CV_ISF1])     # theta (u units)
            nc.vector.tensor_scalar_mul(negth2[:], tmp_a[:], -1.0)

            # =============== conv2 ===============
            def prep2(n, s, part):
                r0, r1_ = (0, 32) if part == 0 else (32, 56)
                for g in range(2):
                    nc.scalar.activation(
                        s_plane(s, g)[:, 1 + r0:1 + r1_, 1:57],
                        u1[n][:, g, r0:r1_, :],
                        AF.Sign, bias=negth2[:, g:g + 1], scale=1.0,
                    )

            u2 = conv_pass(1, w2t, st3, prep2, u2_pool, CV_A2)

            # u1 fully consumed by prep2; release its pool
            es_u1.close()

            # reduce st3 -> r3, AllGather -> g3t
            r3 = cpool.tile([128, 4], F32, tag="r3")
            nc.vector.reduce_sum(
                r3[:].rearrange("p (a b) -> p a b", b=1),
                st3[:].rearrange("p (kc t) -> p kc t", t=28),
                axis=mybir.AxisListType.X,
            )
            ar3_i = dram_pool.tile([128, 4], F32, tag="ar3_i")
            ar3_o = dram_pool.tile([NCORES, 128, 4], F32, tag="ar3_o", addr_space="Shared")
            nc.sync.dma_start(ar3_i[:], r3[:])
            nc.gpsimd.collective_compute(
                "AllGather", OP.bypass, replica_groups=[list(range(NCORES))],
                ins=[ar3_i[:].opt()], outs=[ar3_o[:].opt()],
            )
            gth3 = cpool.tile([128, 4, NCORES], F32, tag="gth3")
            nc.scalar.dma_start(gth3[:], ar3_o[:].rearrange("r p k -> p k r"))
            nc.vector.reduce_sum(
                g3t[:].rearrange("p (a b) -> p a b", b=1), gth3[:],
                axis=mybir.AxisListType.X,
            )

            # ---- BN3 affine: K = g3*sf2*rstd3, D = b3 - m3*g3*rstd3 ----
            g3v = g3t[:].rearrange("p (k c) -> p k c", k=2)
            nc.vector.tensor_mul(tmp_a[:], g3v[:, 0], cvec[:, :, CV_SF2N])    # m3
            nc.vector.tensor_mul(tmp_b[:], g3v[:, 1], cvec[:, :, CV_SF2SQN])  # E[p2^2]
            nc.vector.scalar_tensor_tensor(
                tmp_c[:], tmp_a[:], -1.0, tmp_a[:], op0=OP.mult, op1=OP.mult,
            )
            nc.vector.tensor_add(tmp_c[:], tmp_c[:], tmp_b[:])                # v3
            nc.scalar.activation(tmp_b[:], tmp_c[:], AF.Sqrt, bias=cvec[:, 0, CV_EPS:CV_EPS + 1], scale=1.0)  # std3
            nc.vector.reciprocal(tmp_c[:], tmp_b[:])                          # rstd3
            nc.vector.tensor_mul(kvec[:], tmp_c[:], cvec[:, :, CV_G3SF2])     # K
            nc.vector.tensor_mul(tmp_a[:], tmp_a[:], cvec[:, :, CV_G3])       # m3*g3
            nc.vector.tensor_mul(tmp_a[:], tmp_a[:], tmp_c[:])                # m3*g3*rstd3
            nc.vector.tensor_sub(dvec[:], cvec[:, :, CV_B3V], tmp_a[:])       # D

            # ====== phase 3: out = prelu(K*u2 + D + x, a3), fp16 out ======
            with tc.tile_pool(name="ph3", bufs=2) as p3pool:
                for n in range(BL):
                    th = p3pool.tile([128, 2, HW], F16, tag="th", name=f"th_{n}")
                    ot = p3pool.tile([128, 2, HW], F16, tag="ot", name=f"ot_{n}")
                    for g in range(2):
                        nc.scalar.activation(
                            th[:, g, :],
                            u2[n][:, g, :, :].rearrange("p h w -> p (h w)"),
                            AF.Identity,
                            bias=dvec[:, g:g + 1], scale=kvec[:, g:g + 1],
                        )
                    for g in range(2):
                        # w = th + x (2x tt), aw = a3*w (4x ts),
                        # out = max(w, aw) (2x tt) — per-g DVE pipeline
                        nc.vector.tensor_tensor(
                            th[:, g, :], th[:, g, :], x16s[n][:, g, :], op=OP.add,
                        )
                        nc.vector.tensor_scalar(
                            ot[:, g, :], th[:, g, :],
                            cvec[:, g, CV_A3:CV_A3 + 1], None, op0=OP.mult,
                        )
                        nc.vector.tensor_tensor(
                            ot[:, g, :], ot[:, g, :], th[:, g, :], op=OP.max,
                        )
                    nc.sync.dma_start(out_view(n), ot[:])

    nc.compile()
    return nc


def _host_prep(inputs):
    x = np.ascontiguousarray(np.asarray(inputs["x"], dtype=np.float32))
    w1 = np.asarray(inputs["w1"], dtype=np.float32)
    w2 = np.asarray(inputs["w2"], dtype=np.float32)

    def wprep(w):
        ws = np.sign(w).astype(np.float32)  # [co, ci, kh, kw]
        sf = np.abs(w).mean(axis=(1, 2, 3)).astype(np.float32)  # [256]
        arr = np.empty((128, 2, 18, 128), dtype=np.float32)
        for o, (dh, dw) in enumerate(OFFS):
            for cc in range(2):
                t = ws[cc * 128:(cc + 1) * 128, :, dh, dw]  # [m, ci]
                # arr[p, g, blk, m] = t[m, g*128 + p]
                arr[:, :, o * 2 + cc, :] = t.T.reshape(2, 128, 128).transpose(1, 0, 2)
        return arr.reshape(128, 2, 18 * 128).astype(F8NP), sf

    w1s, sf1 = wprep(w1)
    w2s, sf2 = wprep(w2)

    def vec(v):
        return np.asarray(v, dtype=np.float32).reshape(2, 128).T  # [p, g]

    g1v, b1v = inputs["g1"], inputs["b1"]
    g2v, b2v = inputs["g2"], inputs["b2"]
    g3v, b3v = inputs["g3"], inputs["b3"]
    a1, a2, a3 = inputs["a1"], inputs["a2"], inputs["a3"]

    cvec = np.zeros((128, 2, CV_NCOLS), dtype=np.float32)
    cvec[:, :, CV_B1] = vec(np.asarray(b1v) / np.asarray(g1v))
    cvec[:, :, CV_A1] = vec(np.asarray(a1))
    cvec[:, :, CV_SF1N] = vec(sf1 / NTOT_)
    cvec[:, :, CV_SF1SQN] = vec(sf1 * sf1 / NTOT_)
    cvec[:, :, CV_B2] = vec(np.asarray(b2v) / np.asarray(g2v))
    cvec[:, :, CV_ISF1] = vec(1.0 / sf1)
    cvec[:, :, CV_A2] = vec(np.asarray(a2))
    cvec[:, :, CV_SF2N] = vec(sf2 / NTOT_)
    cvec[:, :, CV_SF2SQN] = vec(sf2 * sf2 / NTOT_)
    cvec[:, :, CV_G3SF2] = vec(np.asarray(g3v) * sf2)
    cvec[:, :, CV_G3] = vec(np.asarray(g3v))
    cvec[:, :, CV_B3V] = vec(np.asarray(b3v))
    cvec[:, :, CV_A3] = vec(np.asarray(a3))
    cvec[:, :, CV_EPS] = EPS

    return x, w1s, w2s, cvec


def run(inputs, trace=False):
    x, w1s, w2s, cvec = _host_prep(inputs)
    if "nc" not in _CACHE:
        _CACHE["nc"] = _build()
    nc = _CACHE["nc"]
    in_maps = [
        {"x": x[i * BL:(i + 1) * BL], "w1s": w1s, "w2s": w2s, "cvec": cvec}
        for i in range(NCORES)
    ]
    res = run_bass_kernel_spmd(nc, in_maps, list(range(NCORES)), trace=trace)
    out = np.concatenate([res.results[i]["out"] for i in range(NCORES)], axis=0)
    return out.astype(np.float32), res


def kernel(**inputs):
    out, _ = run(inputs, trace=False)
    return out


if __name__ == "__main__":
    # build-only check
    _build()
    print("BUILD OK")
